# revision 1
# baseline (speedup 1.0000x reference)
"""Trainium2 Bass kernel for nn_CDE: natural-cubic-spline CDE with RK4(3/8) solver.

Strategy:
- Host: fold the spline solve into a fixed (60, 16) matrix C60 over the time
  axis (derived from t alone): every RK-stage derivative dX/dt is a linear
  combination of the 16 knots of x.  Pack only mask-active trajectories,
  pad to 8*Nc (Nc in {128, 256}), shard across 8 cores.
- Device (per core): feature-major MLP (layers 0-2: [feat_part, batch_free]),
  batch-major layer 3 (stationary = activations) with W3 rows permuted
  d-major so the einsum over D becomes fused DVE/GPSIMD scalar_tensor_tensor
  chains with per-partition dX scalars.  PE transpose brings k back to
  feature-major; each stage boundary is a single fused update reading the
  transpose PSUM directly (RK partials precomputed off the critical path).
- Matmuls run in float32r (TF32-class, ~1e-3 end-to-end rel err, full-rate).
"""
import os
import sys
import types

for _p in ("/opt/trn_rl_repo", "/root/.axon_site/_ro/trn_rl_repo"):
    if os.path.isdir(_p) and _p not in sys.path:
        sys.path.insert(0, _p)

# antenv.axon_hooks shim so BASS_TRACE=1 works under axon (missing in image)
if "antenv.axon_hooks" not in sys.modules:
    _m = types.ModuleType("antenv.axon_hooks")
    _hook = [None]

    def _set(hook):
        _hook[0] = hook

    def _get():
        if _hook[0] is None:
            try:
                from trn_agent_boot.trn_boot import _ntff_profile_via_ctypes
                _hook[0] = _ntff_profile_via_ctypes("/opt/axon/libaxon_pjrt.so")
            except Exception:
                pass
        return _hook[0]

    _m.set_axon_ntff_profile_hook = _set
    _m.get_axon_ntff_profile_hook = _get
    sys.modules["antenv.axon_hooks"] = _m

import numpy as np

N_CORES = 8
T, D, E, H = 16, 10, 128, 512
F3 = E * D  # 1280
N_STEPS = T - 1
N_STAGES = 4 * N_STEPS  # 60
SLICES = [(0, 512), (512, 1024), (1024, 1280)]
# einsum d-chains aligned with tanh slices (slice0: d0-3, slice1: d4-7, slice2: d8-9)
CHAIN_A = [0, 1, 2, 3, 8]
CHAIN_B = [4, 5, 6, 7, 9]

last_results = None


def spline_stage_matrix(t):
    """C60 (60,16): row 4j+r maps the 16 knots of one scalar series to the
    spline derivative at RK stage r of step j.  Also returns h (15,)."""
    t = np.asarray(t, np.float64)
    Tn = len(t)
    h = np.diff(t)
    A = np.zeros((Tn, Tn))
    A[0, 0] = 1.0
    A[-1, -1] = 1.0
    for i in range(1, Tn - 1):
        A[i, i - 1] = h[i - 1]
        A[i, i] = 2.0 * (h[i - 1] + h[i])
        A[i, i + 1] = h[i]
    R = np.zeros((Tn, Tn))
    for i in range(1, Tn - 1):
        R[i, i - 1] = 6.0 / h[i - 1]
        R[i, i] = -6.0 / h[i - 1] - 6.0 / h[i]
        R[i, i + 1] = 6.0 / h[i]
    S = np.linalg.solve(A, R)  # M = S @ x  (second derivatives)
    Iden = np.eye(Tn)
    rows = []
    for j in range(Tn - 1):
        hs = h[j]
        for u_frac in (0.0, 1.0 / 3.0, 2.0 / 3.0, 1.0):
            s = t[j + 1] if u_frac == 1.0 else t[j] + u_frac * hs
            i = int(np.clip(np.searchsorted(t, s, side="right") - 1, 0, Tn - 2))
            u = s - t[i]
            b_row = (Iden[i + 1] - Iden[i]) / h[i] - h[i] * (2.0 * S[i] + S[i + 1]) / 6.0
            rows.append(b_row + u * S[i] + (u * u) / (2.0 * h[i]) * (S[i + 1] - S[i]))
    return np.asarray(rows), h


def w3_perm():
    """Permutation so W3p[f'] = W3[e*10+d] with f' = d*128+e (d-major)."""
    fp = np.arange(F3)
    return (fp % E) * D + fp // E


def rk4_weights_sim(x_pack, C60, h, W_embed, b_embed, W0, b0, W1, b1, W2, b2, W3, b3):
    """Numpy simulation of the exact device math."""
    n = x_pack.shape[0]
    dx_all = np.einsum("st,ntd->snd", C60, x_pack)  # (60, n, 10)
    z = x_pack[:, 0, :] @ W_embed.T + b_embed

    def f(zz):
        y = np.maximum(zz @ W0.T + b0, 0)
        y = np.maximum(y @ W1.T + b1, 0)
        y = np.maximum(y @ W2.T + b2, 0)
        y = np.tanh(y @ W3.T + b3)
        return y.reshape(n, E, D)

    for j in range(N_STEPS):
        hs = h[j]
        k1 = np.einsum("ned,nd->ne", f(z), dx_all[4 * j + 0])
        k2 = np.einsum("ned,nd->ne", f(z + hs * k1 / 3.0), dx_all[4 * j + 1])
        k3 = np.einsum("ned,nd->ne", f(z + hs * (k2 - k1 / 3.0)), dx_all[4 * j + 2])
        k4 = np.einsum("ned,nd->ne", f(z + hs * (k1 - k2 + k3)), dx_all[4 * j + 3])
        z = z + hs * (k1 + 3.0 * (k2 + k3) + k4) / 8.0
    return z


def build_bass(Nc, dt_name, h, dve_writes_dt=True, gps_einsum=True):
    """Build the per-core SPMD Bass program (fully unrolled 60 stages)."""
    import concourse.bass as bass
    import concourse.bacc as bacc
    import concourse.tile as tile
    import concourse.mybir as mybir
    from concourse.masks import make_identity

    F32 = mybir.dt.float32
    F32R = mybir.dt.float32r
    BF16 = mybir.dt.bfloat16
    AF = mybir.ActivationFunctionType
    ALU = mybir.AluOpType
    DT = {"f32r": F32R, "bf16": BF16}[dt_name]

    NT = Nc // 128
    nc = bacc.Bacc("TRN2", target_bir_lowering=False)

    d_xbyd = nc.dram_tensor("x_byd", [T, D, Nc], F32, kind="ExternalInput")
    d_xt0 = nc.dram_tensor("x_t0", [D, Nc], F32, kind="ExternalInput")
    d_c60 = nc.dram_tensor("c60t", [T, N_STAGES], F32, kind="ExternalInput")
    d_wemb = nc.dram_tensor("w_embt", [D, E], F32, kind="ExternalInput")
    d_bemb = nc.dram_tensor("b_emb", [E, 1], F32, kind="ExternalInput")
    d_w0 = nc.dram_tensor("w0t", [E, H], DT, kind="ExternalInput")
    d_w1 = nc.dram_tensor("w1t", [H, H], DT, kind="ExternalInput")
    d_w2 = nc.dram_tensor("w2t", [H, H], DT, kind="ExternalInput")
    d_w3 = nc.dram_tensor("w3pt", [H, F3], DT, kind="ExternalInput")
    d_b012 = nc.dram_tensor("b012", [E, 12], F32, kind="ExternalInput")
    d_ones4 = nc.dram_tensor("ones4", [4, E], F32R, kind="ExternalInput")
    d_b3p4 = nc.dram_tensor("b3p4", [4, F3], F32R, kind="ExternalInput")
    d_out = nc.dram_tensor("zout", [E, Nc], F32, kind="ExternalOutput")

    with tile.TileContext(nc) as tc:
        with (
            tc.tile_pool(name="wpool", bufs=1) as wpool,
            tc.tile_pool(name="xpool", bufs=1) as xpool,
            tc.tile_pool(name="apool", bufs=2) as apool,
            tc.tile_pool(name="pmlp", bufs=(3 if NT == 1 else 2), space="PSUM") as pmlp,
            tc.tile_pool(name="p3p", bufs=2, space="PSUM") as p3p,
            tc.tile_pool(name="ptrp", bufs=2, space="PSUM") as ptrp,
        ):
            # ---- load constants / weights
            w0t = wpool.tile([E, H], DT, tag="w0t")
            nc.sync.dma_start(out=w0t, in_=d_w0[:, :])
            w1k = [wpool.tile([128, H], DT, tag=f"w1k{k}", name=f"w1k{k}")
                   for k in range(4)]
            w2k = [wpool.tile([128, H], DT, tag=f"w2k{k}", name=f"w2k{k}")
                   for k in range(4)]
            w3k = [wpool.tile([128, F3], DT, tag=f"w3k{k}", name=f"w3k{k}")
                   for k in range(4)]
            for k in range(4):
                nc.sync.dma_start(out=w1k[k], in_=d_w1[128 * k:128 * (k + 1), :])
                nc.sync.dma_start(out=w2k[k], in_=d_w2[128 * k:128 * (k + 1), :])
                nc.sync.dma_start(out=w3k[k], in_=d_w3[128 * k:128 * (k + 1), :])
            b012 = wpool.tile([E, 12], F32, tag="b012")
            nc.sync.dma_start(out=b012, in_=d_b012[:, :])
            bemb = wpool.tile([E, 1], F32, tag="bemb")
            nc.sync.dma_start(out=bemb, in_=d_bemb[:, :])
            ones4 = wpool.tile([4, E], F32R, tag="ones4")
            nc.sync.dma_start(out=ones4, in_=d_ones4[:, :])
            b3p4 = wpool.tile([4, F3], F32R, tag="b3p4")
            nc.sync.dma_start(out=b3p4, in_=d_b3p4[:, :])
            wembt = wpool.tile([D, E], F32, tag="wembt")
            nc.sync.dma_start(out=wembt, in_=d_wemb[:, :])
            xbyd = xpool.tile([T, D, Nc], F32, tag="xbyd")
            nc.sync.dma_start(out=xbyd, in_=d_xbyd[:, :, :])
            xt0 = xpool.tile([D, Nc], F32, tag="xt0")
            nc.sync.dma_start(out=xt0, in_=d_xt0[:, :])
            c60 = xpool.tile([T, N_STAGES], F32, tag="c60")
            nc.sync.dma_start(out=c60, in_=d_c60[:, :])
            ident = wpool.tile([128, 128], F32, tag="ident")
            make_identity(nc, ident)
            ident_r = wpool.tile([128, 128], F32R, tag="ident_r")
            nc.scalar.activation(ident_r, ident, AF.Identity, bias=0.0, scale=1.0)

            # ---- spline: DXb[nt][:, d, s] = dX/dt (traj partition, coord d, stage s)
            DXb = [xpool.tile([128, D, N_STAGES], F32, tag=f"dxb{nt}", name=f"dxb{nt}")
                   for nt in range(NT)]
            for nt in range(NT):
                for d in range(D):
                    pdx = ptrp.tile([128, 256], F32, tag="ptr", name=f"pdx{nt}_{d}")
                    nc.tensor.matmul(pdx[:, 0:N_STAGES],
                                     xbyd[:, d, 128 * nt:128 * (nt + 1)],
                                     c60[:, :], start=True, stop=True)
                    nc.vector.tensor_copy(DXb[nt][:, d, :], pdx[:, 0:N_STAGES])

            # ---- embed: z0 = W_embed @ x(t0) + b, split per n-chunk
            pemb = ptrp.tile([128, 256], F32, tag="ptr", name="pemb")
            nc.tensor.matmul(pemb[:, 0:Nc], wembt[:, :], xt0[:, :],
                             start=True, stop=True)
            z = apool.tile([E, Nc], F32, tag="z", name="z0")
            nc.scalar.activation(z, pemb[:, 0:Nc], AF.Identity,
                                 bias=bemb[:, :], scale=1.0)

            def relu(eng, out_ap, in_ap, bias_ap):
                if eng == "act":
                    nc.scalar.activation(out_ap, in_ap, AF.Relu,
                                         bias=bias_ap, scale=1.0)
                else:
                    nc.vector.tensor_scalar(out=out_ap, in0=in_ap,
                                            scalar1=bias_ap, scalar2=0.0,
                                            op0=ALU.add, op1=ALU.max)

            def stt(eng, out_ap, in0_ap, scalar, in1_ap):
                eng.scalar_tensor_tensor(
                    out=out_ap, in0=in0_ap, scalar=scalar, in1=in1_ap,
                    op0=ALU.mult, op1=ALU.add)

            # relu engine by m-chunk: first chunks on DVE (lower latency),
            # later chunks on ACT (parallel engine)
            R_ENG = ["dve", "act", "dve", "act"] if dve_writes_dt else ["act"] * 4

            def mlp_and_k(s, zin, ks_psum_out):
                """Vector-field eval; per-n-chunk pipelines interleaved so each
                half's relu/tanh/einsum latency hides under the other half's
                matmuls (keeps the PE gap-free and the HAM clock warm)."""
                WN = 128 * NT
                p0 = pmlp.tile([128, 4, WN], F32, tag="pmlp", name=f"p0_{s}")
                for m in range(4):
                    nc.tensor.matmul(p0[:, m, :], w0t[:, 128 * m:128 * (m + 1)],
                                     zin[:, :], start=True, stop=True)
                y0 = apool.tile([128, 4, WN], DT, tag="y0", name=f"y0_{s}")
                for m in range(4):
                    relu(R_ENG[m], y0[:, m, :], p0[:, m, :], b012[:, m:m + 1])
                p1 = pmlp.tile([128, 4, WN], F32, tag="pmlp", name=f"p1_{s}")
                for m in range(4):
                    for k in range(4):
                        nc.tensor.matmul(p1[:, m, :],
                                         w1k[k][:, 128 * m:128 * (m + 1)],
                                         y0[:, k, :], start=(k == 0), stop=(k == 3))
                y1 = apool.tile([128, 4, WN], DT, tag="y1", name=f"y1_{s}")
                for m in range(4):
                    relu(R_ENG[m], y1[:, m, :], p1[:, m, :], b012[:, 4 + m:5 + m])
                p2 = pmlp.tile([128, 4, WN], F32, tag="pmlp", name=f"p2_{s}")
                for m in range(4):
                    for k in range(4):
                        nc.tensor.matmul(p2[:, m, :],
                                         w2k[k][:, 128 * m:128 * (m + 1)],
                                         y1[:, k, :], start=(k == 0), stop=(k == 3))
                y2 = apool.tile([128, 4, WN], DT, tag="y2", name=f"y2_{s}")
                for m in range(4):
                    relu(R_ENG[m], y2[:, m, :], p2[:, m, :], b012[:, 8 + m:9 + m])

                tmps = []
                for nt in range(NT):
                    y3t = apool.tile([128, D, E], F32, tag=f"y3t{nt}",
                                     name=f"y3t_{s}_{nt}")
                    tmp = apool.tile([128, D, E], F32R, tag=f"etmp{nt}",
                                     name=f"etmp_{s}_{nt}")
                    dxap = DXb[nt]
                    dstride = dxap.ap[1][0]
                    for (sl0, sl1) in SLICES:
                        w = sl1 - sl0
                        d0, nd = sl0 // E, w // E
                        p3 = p3p.tile([128, 512], F32, tag="p3",
                                      name=f"p3_{s}_{nt}_{sl0}")
                        nc.tensor.matmul(p3[:, 0:w], ones4[:, :],
                                         b3p4[:, sl0:sl1], start=True, stop=False)
                        for k in range(4):
                            nc.tensor.matmul(p3[:, 0:w],
                                             y2[:, k, 128 * nt:128 * (nt + 1)],
                                             w3k[k][:, sl0:sl1],
                                             start=False, stop=(k == 3))
                        nc.scalar.activation(y3t[:, d0:d0 + nd, :], p3[:, 0:w],
                                             AF.Tanh)
                        dx_b = bass.AP(
                            tensor=dxap.tensor,
                            offset=dxap.offset + d0 * dstride + s,
                            ap=[dxap.ap[0], [dstride, nd], [0, E]])
                        nc.vector.tensor_tensor(
                            out=tmp[:, d0:d0 + nd, :],
                            in0=y3t[:, d0:d0 + nd, :], in1=dx_b, op=ALU.mult)
                    tmps.append(tmp)
                # reduce over d on PE: accumulating tmp_d.T @ I matmuls.
                # Emitted after BOTH halves' L3 so each reduce has runway.
                for nt in range(NT):
                    ktr = ptrp.tile([128, 256], F32, tag="ptr", name=f"ktr_{s}_{nt}")
                    for d in range(D):
                        nc.tensor.matmul(ktr[:, 0:128], tmps[nt][:, d, :],
                                         ident_r[:, :],
                                         start=(d == 0), stop=(d == D - 1))
                    ks_psum_out.append(ktr)

            def pair(tag, name, dt):
                return [apool.tile([E, 128], dt, tag=f"{tag}_{nt}",
                                   name=f"{name}_{nt}") for nt in range(NT)]

            def wide(tag, name, dt):
                return apool.tile([E, 128 * NT], dt, tag=tag, name=name)

            def half(ap, nt):
                return ap[:, 128 * nt:128 * (nt + 1)]

            # RK4 (3/8 rule); per-half fused updates reading transpose PSUM.
            for j in range(N_STEPS):
                hs = float(h[j])
                last_step = j == N_STEPS - 1

                z_rhs = wide("zrhs", f"zrhs_{j}", DT)
                nc.scalar.activation(z_rhs, z, AF.Identity, bias=0.0, scale=1.0)
                kp1 = []
                mlp_and_k(4 * j + 0, z_rhs, kp1)
                zin2 = wide("zs", f"z2in_{j}", DT)
                for nt in range(NT):
                    stt(nc.vector, half(zin2, nt), kp1[nt][:, 0:128], hs / 3.0,
                        half(z, nt))
                k1 = wide("k1", f"k1_{j}", F32)
                for nt in range(NT):
                    nc.vector.tensor_copy(half(k1, nt), kp1[nt][:, 0:128])
                zpart3 = wide("zpart3", f"zpart3_{j}", F32)
                stt(nc.vector, zpart3, k1, -hs / 3.0, z)

                kp2 = []
                mlp_and_k(4 * j + 1, zin2, kp2)
                zin3 = wide("zs", f"z3in_{j}", DT)
                for nt in range(NT):
                    stt(nc.vector, half(zin3, nt), kp2[nt][:, 0:128], hs,
                        half(zpart3, nt))
                k2 = wide("k2", f"k2_{j}", F32)
                for nt in range(NT):
                    nc.vector.tensor_copy(half(k2, nt), kp2[nt][:, 0:128])
                k12d = wide("k12d", f"k12d_{j}", F32)
                stt(nc.vector, k12d, k2, -1.0, k1)
                zpart4 = wide("zpart4", f"zpart4_{j}", F32)
                stt(nc.vector, zpart4, k12d, hs, z)

                kp3 = []
                mlp_and_k(4 * j + 2, zin3, kp3)
                zin4 = wide("zs", f"z4in_{j}", DT)
                for nt in range(NT):
                    stt(nc.vector, half(zin4, nt), kp3[nt][:, 0:128], hs,
                        half(zpart4, nt))
                k3 = wide("k3", f"k3_{j}", F32)
                for nt in range(NT):
                    nc.vector.tensor_copy(half(k3, nt), kp3[nt][:, 0:128])
                s1 = wide("s1", f"s1_{j}", F32)
                nc.vector.tensor_tensor(out=s1, in0=k2, in1=k3, op=ALU.add)
                s2 = wide("s2", f"s2_{j}", F32)
                stt(nc.vector, s2, s1, 3.0, k1)
                zpre = wide("zpre", f"zpre_{j}", F32)
                stt(nc.vector, zpre, s2, hs / 8.0, z)

                kp4 = []
                mlp_and_k(4 * j + 3, zin4, kp4)
                znew = wide("zfin" if last_step else "z", f"z_{j + 1}", F32)
                for nt in range(NT):
                    stt(nc.vector, half(znew, nt), kp4[nt][:, 0:128], hs / 8.0,
                        half(zpre, nt))
                z = znew

            nc.sync.dma_start(out=d_out[:, :], in_=z)
    nc.finalize()
    return nc


def _prep_host(t, x, mask, W_embed, b_embed, W0, b0, W1, b1, W2, b2, W3, b3,
               dt_name):
    import ml_dtypes
    wdt = {"f32r": np.float32, "bf16": ml_dtypes.bfloat16}[dt_name]

    t = np.asarray(t, np.float32)
    x = np.asarray(x, np.float32)
    mask = np.asarray(mask)
    B, Amax = mask.shape
    N = B * Amax

    C60, h = spline_stage_matrix(t)
    idx = np.flatnonzero(mask.ravel())
    nact = len(idx)
    Nc = 128 if nact <= N_CORES * 128 else 256
    total = N_CORES * Nc
    pad = np.full(total, idx[0] if nact else 0, dtype=np.int64)
    pad[:nact] = idx
    xp = x.reshape(N, T, D)[pad]  # (total, 16, 10)

    perm = w3_perm()
    shared = dict(
        c60t=np.ascontiguousarray(C60.T.astype(np.float32)),
        w_embt=np.ascontiguousarray(np.asarray(W_embed, np.float32).T),
        b_emb=np.asarray(b_embed, np.float32).reshape(E, 1),
        w0t=np.ascontiguousarray(np.asarray(W0).T).astype(wdt),
        w1t=np.ascontiguousarray(np.asarray(W1).T).astype(wdt),
        w2t=np.ascontiguousarray(np.asarray(W2).T).astype(wdt),
        w3pt=np.ascontiguousarray(np.asarray(W3)[perm].T).astype(wdt),
        b012=np.stack([np.asarray(b, np.float32)[m * 128:(m + 1) * 128]
                       for b in (b0, b1, b2) for m in range(4)],
                      axis=1).astype(np.float32),
        ones4=np.full((4, E), 0.25, np.float32),
        b3p4=np.tile(np.asarray(b3, np.float32)[perm][None, :], (4, 1)),
    )
    in_maps = []
    for c in range(N_CORES):
        xc = xp[c * Nc:(c + 1) * Nc]  # (Nc, 16, 10)
        in_maps.append(dict(
            x_byd=np.ascontiguousarray(xc.transpose(1, 2, 0)),  # (16,10,Nc)
            x_t0=np.ascontiguousarray(xc[:, 0, :].T),           # (10,Nc)
            **shared,
        ))
    return in_maps, pad, nact, Nc, h, C60, xp


def _enable_ldw_opt():
    """Flip walrus --enable-ldw-opt to true (background weight loads)."""
    from concourse import bass_utils as _bu
    if getattr(_bu, "_ldwopt_patched", False):
        return
    _orig = _bu.run_command

    def _run2(argv, **kw):
        argv = ["--enable-ldw-opt=true" if a == "--enable-ldw-opt=false" else a
                for a in argv]
        return _orig(argv, **kw)

    _bu.run_command = _run2
    _bu._ldwopt_patched = True


def kernel(t, x, mask, W_embed, b_embed, W0, b0, W1, b1, W2, b2, W3, b3):
    global last_results
    from concourse import bass_utils
    if os.environ.get("KERNEL_LDWOPT", "0") == "1":
        _enable_ldw_opt()

    dt_name = os.environ.get("KERNEL_DT", "f32r")
    gps_einsum = os.environ.get("KERNEL_GPS", "1") == "1"
    mask = np.asarray(mask)
    B, Amax = mask.shape
    N = B * Amax

    in_maps, pad, nact, Nc, h, _, _ = _prep_host(
        t, x, mask, W_embed, b_embed, W0, b0, W1, b1, W2, b2, W3, b3, dt_name)

    res = None
    err = None
    for dve_dt in (True, False):
        try:
            nc = build_bass(Nc, dt_name, h, dve_writes_dt=dve_dt,
                            gps_einsum=gps_einsum)
            res = bass_utils.run_bass_kernel_spmd(
                nc, in_maps, core_ids=list(range(N_CORES)))
            break
        except Exception as e:  # retry with conservative engine config
            err = e
            continue
    if res is None:
        raise err
    last_results = res

    zall = np.concatenate([r["zout"].T for r in res.results], 0)  # (total, E)
    out = np.zeros((N, E), np.float32)
    out[pad[:nact]] = zall[:nact]
    return out.reshape(B, Amax, E)



# revision 2
# speedup vs baseline: 1.0392x; 1.0392x over previous
"""Trainium2 Bass kernel for nn_CDE: bf16 feature-major pipeline.

Design vs v1:
- Nc = ceil(nact/8) rounded up (132 for the 1044-active case) instead of 256:
  streams scale with actual row count.
- All matmuls bf16 (1 cyc/row at any free width; f32r pays 4x below 256).
- L3 feature-major per-d (W3 stationary, d-major permutation): streams scale
  with Nc, bias folds into the per-partition tanh bias, and the einsum
  reduction moves to one strided DVE tensor_reduce -- no transpose matmuls,
  no bias matmuls.
- dX/dt rows are PE-broadcast into PSUM ([1,128] ones stationary) per stage;
  emitted as next-stage filler behind L3 so the PE stays busy during the
  tanh/mult/reduce tail.
- RK z-update restructured: exactly one fused scalar_tensor_tensor on the
  critical path per sub-stage; all other partials run off-path on Pool.
- PSUM: tag "pp" (mlp m-chunks + L3 per-d, bufs=4) + tag "pdx" (dx broadcast,
  bufs=4) = 8 banks.
"""
import os
import sys
import types

for _p in ("/opt/trn_rl_repo", "/root/.axon_site/_ro/trn_rl_repo"):
    if os.path.isdir(_p) and _p not in sys.path:
        sys.path.insert(0, _p)

if "antenv.axon_hooks" not in sys.modules:
    _m = types.ModuleType("antenv.axon_hooks")
    _hook = [None]

    def _set(hook):
        _hook[0] = hook

    def _get():
        if _hook[0] is None:
            try:
                from trn_agent_boot.trn_boot import _ntff_profile_via_ctypes
                _hook[0] = _ntff_profile_via_ctypes("/opt/axon/libaxon_pjrt.so")
            except Exception:
                pass
        return _hook[0]

    _m.set_axon_ntff_profile_hook = _set
    _m.get_axon_ntff_profile_hook = _get
    sys.modules["antenv.axon_hooks"] = _m

import numpy as np

N_CORES = 8
T, D, E, H = 16, 10, 128, 512
F3 = E * D
N_STEPS = T - 1
N_STAGES = 4 * N_STEPS  # 60
# dx-broadcast d-groups per stage (each tile <= 1 psum bank at Nc<=160)
DX_GROUPS = [(0, 3), (3, 3), (6, 3), (9, 1)]

last_results = None


def spline_stage_matrix(t):
    """C60 (60,16): row 4j+r maps 16 knots of a scalar series to the spline
    derivative at RK stage r of step j.  Also returns h (15,)."""
    t = np.asarray(t, np.float64)
    Tn = len(t)
    h = np.diff(t)
    A = np.zeros((Tn, Tn))
    A[0, 0] = 1.0
    A[-1, -1] = 1.0
    for i in range(1, Tn - 1):
        A[i, i - 1] = h[i - 1]
        A[i, i] = 2.0 * (h[i - 1] + h[i])
        A[i, i + 1] = h[i]
    R = np.zeros((Tn, Tn))
    for i in range(1, Tn - 1):
        R[i, i - 1] = 6.0 / h[i - 1]
        R[i, i] = -6.0 / h[i - 1] - 6.0 / h[i]
        R[i, i + 1] = 6.0 / h[i]
    S = np.linalg.solve(A, R)
    Iden = np.eye(Tn)
    rows = []
    for j in range(Tn - 1):
        hs = h[j]
        for u_frac in (0.0, 1.0 / 3.0, 2.0 / 3.0, 1.0):
            s = t[j + 1] if u_frac == 1.0 else t[j] + u_frac * hs
            i = int(np.clip(np.searchsorted(t, s, side="right") - 1, 0, Tn - 2))
            u = s - t[i]
            b_row = (Iden[i + 1] - Iden[i]) / h[i] - h[i] * (2.0 * S[i] + S[i + 1]) / 6.0
            rows.append(b_row + u * S[i] + (u * u) / (2.0 * h[i]) * (S[i + 1] - S[i]))
    return np.asarray(rows), h


def w3_perm():
    """Permutation so W3p[f'] = W3[e*10+d] with f' = d*128+e (d-major)."""
    fp = np.arange(F3)
    return (fp % E) * D + fp // E


def build_bass2(Nc, h):
    import concourse.bass as bass
    import concourse.bacc as bacc
    import concourse.tile as tile
    import concourse.mybir as mybir

    F32 = mybir.dt.float32
    F32R = mybir.dt.float32r
    BF16 = mybir.dt.bfloat16
    AF = mybir.ActivationFunctionType
    ALU = mybir.AluOpType
    AX = mybir.AxisListType

    nc = bacc.Bacc("TRN2", target_bir_lowering=False)

    d_xt0 = nc.dram_tensor("x_t0", [128, Nc], BF16, kind="ExternalInput")
    d_dxh = nc.dram_tensor("dxh", [D, N_STAGES * Nc], BF16, kind="ExternalInput")
    d_wemb = nc.dram_tensor("w_embt", [128, E], BF16, kind="ExternalInput")
    d_bemb = nc.dram_tensor("b_emb", [E, 1], F32, kind="ExternalInput")
    d_w0 = nc.dram_tensor("w0t", [E, H], BF16, kind="ExternalInput")
    d_w1 = nc.dram_tensor("w1t", [H, H], BF16, kind="ExternalInput")
    d_w2 = nc.dram_tensor("w2t", [H, H], BF16, kind="ExternalInput")
    d_w3 = nc.dram_tensor("w3pt", [H, F3], BF16, kind="ExternalInput")
    d_b012 = nc.dram_tensor("b012", [E, 12], F32, kind="ExternalInput")
    d_b3seg = nc.dram_tensor("b3seg", [128, 4 * E], BF16, kind="ExternalInput")
    d_sel3 = nc.dram_tensor("sel3", [128, 3 * Nc], BF16, kind="ExternalInput")
    d_out = nc.dram_tensor("zout", [E, Nc], F32, kind="ExternalOutput")

    with tile.TileContext(nc) as tc:
        with (
            tc.tile_pool(name="wpool", bufs=1) as wpool,
            tc.tile_pool(name="apool", bufs=2) as apool,
            tc.tile_pool(name="ppool", bufs=4, space="PSUM") as ppool,
        ):
            # ---- weights / constants
            w0t = wpool.tile([E, H], BF16, tag="w0t")
            nc.sync.dma_start(out=w0t, in_=d_w0[:, :])
            w1k = [wpool.tile([128, H], BF16, tag=f"w1k{k}", name=f"w1k{k}")
                   for k in range(4)]
            w2k = [wpool.tile([128, H], BF16, tag=f"w2k{k}", name=f"w2k{k}")
                   for k in range(4)]
            w3k = [wpool.tile([128, F3], BF16, tag=f"w3k{k}", name=f"w3k{k}")
                   for k in range(4)]
            for k in range(4):
                nc.sync.dma_start(out=w1k[k], in_=d_w1[128 * k:128 * (k + 1), :])
                nc.sync.dma_start(out=w2k[k], in_=d_w2[128 * k:128 * (k + 1), :])
                nc.sync.dma_start(out=w3k[k], in_=d_w3[128 * k:128 * (k + 1), :])
            b012 = wpool.tile([E, 12], F32, tag="b012")
            nc.sync.dma_start(out=b012, in_=d_b012[:, :])
            b3seg = wpool.tile([128, 4 * E], BF16, tag="b3seg")
            nc.sync.dma_start(out=b3seg, in_=d_b3seg[:, :])
            sel3 = wpool.tile([128, 3 * Nc], BF16, tag="sel3")
            nc.sync.dma_start(out=sel3, in_=d_sel3[:, :])
            bemb = wpool.tile([E, 1], F32, tag="bemb")
            nc.sync.dma_start(out=bemb, in_=d_bemb[:, :])
            wembt = wpool.tile([128, E], BF16, tag="wembt")
            nc.sync.dma_start(out=wembt, in_=d_wemb[:, :])
            xt0 = wpool.tile([128, Nc], BF16, tag="xt0")
            nc.sync.dma_start(out=xt0, in_=d_xt0[:, :])

            # ---- embed: z0 = W_embed @ x(t0) + b
            pemb = ppool.tile([E, Nc], F32, tag="pp", name="pemb")
            nc.tensor.matmul(pemb, wembt[:, :], xt0[:, :], start=True, stop=True)
            z = apool.tile([E, Nc], F32, tag="z", name="z0")
            nc.scalar.activation(z, pemb, AF.Identity, bias=bemb[:, :], scale=1.0)
            zin = apool.tile([E, Nc], BF16, tag="zin", name="zin0")
            nc.scalar.activation(zin, pemb, AF.Identity, bias=bemb[:, :], scale=1.0)

            # dx rows replicated across partitions via DMA (idle engines)
            dxb_tiles = {}
            l3_tiles = {}

            def emit_l3_bias(s):
                tiles = []
                for gi, (d0, nd) in enumerate(DX_GROUPS):
                    p3 = ppool.tile([128, nd, Nc], F32, tag="pl3", bufs=4,
                                    name=f"p3_{s}_{d0}")
                    nc.tensor.matmul(p3, b3seg[:, gi * E:(gi + 1) * E],
                                     sel3[:, 0:nd * Nc], start=True,
                                     stop=False, skip_group_check=True)
                    tiles.append(p3)
                l3_tiles[s] = tiles

            def emit_bcast(s):
                dxS = wpool.tile([E, D, Nc], BF16, tag="dxS", bufs=3,
                                 name=f"dxS_{s}")
                dap = d_dxh[:, :]
                src_bc = bass.AP(
                    tensor=dap.tensor,
                    offset=s * Nc,
                    ap=[[0, E], [N_STAGES * Nc, D], [1, Nc]])
                nc.sync.dma_start(out=dxS, in_=src_bc)
                dxb_tiles[s] = dxS

            def dxb_ap(s, d):
                return dxb_tiles[s][:, d, :]

            def relu(eng, out_ap, in_ap, bias_ap):
                if eng == "act":
                    nc.scalar.activation(out_ap, in_ap, AF.Relu,
                                         bias=bias_ap, scale=1.0)
                else:
                    e = nc.vector if eng == "dve" else nc.gpsimd
                    e.tensor_scalar(out=out_ap, in0=in_ap,
                                    scalar1=bias_ap, scalar2=0.0,
                                    op0=ALU.add, op1=ALU.max)

            def stt(eng, out_ap, in0_ap, scalar, in1_ap):
                e = nc.vector if eng == "dve" else nc.gpsimd
                e.scalar_tensor_tensor(out=out_ap, in0=in0_ap, scalar=scalar,
                                       in1=in1_ap, op0=ALU.mult, op1=ALU.add)

            RELU_ENG = [["act", "dve", "act", "dve"],
                        ["dve", "act", "dve", "act"],
                        ["act", "dve", "act", "dve"]]
            MULT_ENG = ["pool", "pool", "pool", "pool", "pool",
                        "pool", "pool", "pool", "dve", "dve"]

            emit_bcast(0)
            emit_l3_bias(0)

            def vf_stage(s, zin_ap):
                """One vector-field eval; returns k_s [E, Nc] f32."""
                # L0
                y0 = apool.tile([128, 4, Nc], BF16, tag="y0", name=f"y0_{s}")
                p0s = []
                for m in range(4):
                    p0 = ppool.tile([128, Nc], F32, tag="pp", name=f"p0_{s}_{m}")
                    nc.tensor.matmul(p0, w0t[:, 128 * m:128 * (m + 1)],
                                     zin_ap, start=True, stop=True)
                    p0s.append(p0)
                for m in range(4):
                    relu(RELU_ENG[0][m], y0[:, m, :], p0s[m], b012[:, m:m + 1])
                # L1
                y1 = apool.tile([128, 4, Nc], BF16, tag="y1", name=f"y1_{s}")
                for m in range(4):
                    p1 = ppool.tile([128, Nc], F32, tag="pp", name=f"p1_{s}_{m}")
                    for k in range(4):
                        nc.tensor.matmul(p1, w1k[k][:, 128 * m:128 * (m + 1)],
                                         y0[:, k, :], start=(k == 0), stop=(k == 3))
                    relu(RELU_ENG[1][m], y1[:, m, :], p1, b012[:, 4 + m:5 + m])
                # L2
                y2 = apool.tile([128, 4, Nc], BF16, tag="y2", name=f"y2_{s}")
                for m in range(4):
                    p2 = ppool.tile([128, Nc], F32, tag="pp", name=f"p2_{s}_{m}")
                    for k in range(4):
                        nc.tensor.matmul(p2, w2k[k][:, 128 * m:128 * (m + 1)],
                                         y1[:, k, :], start=(k == 0), stop=(k == 3))
                    relu(RELU_ENG[2][m], y2[:, m, :], p2, b012[:, 8 + m:9 + m])
                # L3 per 3-d group: bias matmul seeds psum, 4k accumulate,
                # one tanh per group into contiguous y3all
                y3all = apool.tile([128, D, Nc], BF16, tag="y3a", name=f"y3a_{s}")
                mgs = []
                for gi, (d0, nd) in enumerate(DX_GROUPS):
                    p3 = l3_tiles[s][gi]
                    for i in range(nd):
                        d = d0 + i
                        for k in range(4):
                            nc.tensor.matmul(p3[:, i, :],
                                             w3k[k][:, 128 * d:128 * (d + 1)],
                                             y2[:, k, :], start=False,
                                             stop=(k == 3), skip_group_check=True)
                    nc.scalar.activation(y3all[:, d0:d0 + nd, :], p3, AF.Tanh)
                    # einsum partials as soon as each tanh lands (DVE):
                    # mg = y3*dx for this group; running sums off the tail
                    dxS = dxb_tiles[s]
                    mg = apool.tile([128, nd, Nc], BF16, tag=f"mg{gi}",
                                    name=f"mg_{s}_{gi}")
                    nc.vector.tensor_tensor(out=mg, in0=y3all[:, d0:d0 + nd, :],
                                            in1=dxS[:, d0:d0 + nd, :],
                                            op=ALU.mult)
                    mgs.append(mg)
                    if gi == 1:
                        s01 = apool.tile([128, 3, Nc], F32, tag="s01",
                                         name=f"s01_{s}")
                        nc.vector.tensor_tensor(out=s01, in0=mgs[0], in1=mgs[1],
                                                op=ALU.add)
                    elif gi == 2:
                        u3 = apool.tile([128, 3, Nc], F32, tag="u3",
                                        name=f"u3_{s}")
                        nc.vector.tensor_tensor(out=u3, in0=s01, in1=mgs[2],
                                                op=ALU.add)
                        v1 = apool.tile([128, Nc], F32, tag="v1", name=f"v1_{s}")
                        nc.vector.tensor_tensor(out=v1, in0=u3[:, 0, :],
                                                in1=u3[:, 1, :], op=ALU.add)
                        v2 = apool.tile([128, Nc], F32, tag="v2", name=f"v2_{s}")
                        nc.vector.tensor_tensor(out=v2, in0=v1, in1=u3[:, 2, :],
                                                op=ALU.add)
                # PE fillers for the tail: next stage's dx DMA + L3 bias seeds
                if s + 1 < N_STAGES:
                    emit_bcast(s + 1)
                    emit_l3_bias(s + 1)
                return v2, mgs[3][:, 0, :]

            def tail(s, v2, mg3, coef, zbase, out_dt, out_tag, name):
                """zout = zbase + coef*(v2 + mg3) with only the mg3 stt on the
                critical path; also returns k = v2 + mg3 (off-crit)."""
                w = apool.tile([E, Nc], F32, tag="w", name=f"w_{s}")
                stt("dve", w, v2, coef, zbase)        # off-tail (v2 early)
                zo = apool.tile([E, Nc], out_dt, tag=out_tag, name=name)
                stt("dve", zo, mg3, coef, w)          # on-crit
                k_s = apool.tile([E, Nc], F32,
                                 tag=("k1" if s % 4 == 0 else "ks"),
                                 name=f"k_{s}")
                nc.vector.tensor_tensor(out=k_s, in0=v2, in1=mg3, op=ALU.add)
                return zo, k_s

            for j in range(N_STEPS):
                hs = float(h[j])
                last = j == N_STEPS - 1

                v2, mg3 = vf_stage(4 * j + 0, zin)
                zin2, k1 = tail(4 * j, v2, mg3, hs / 3.0, z, BF16, "zin",
                                f"zin2_{j}")
                zpart3 = apool.tile([E, Nc], F32, tag="zp3", name=f"zp3_{j}")
                stt("dve", zpart3, k1, -hs / 3.0, z)
                zacc = apool.tile([E, Nc], F32, tag="za", name=f"za1_{j}")
                stt("dve", zacc, k1, hs / 8.0, z)

                v2, mg3 = vf_stage(4 * j + 1, zin2)
                zin3, k2 = tail(4 * j + 1, v2, mg3, hs, zpart3, BF16, "zin",
                                f"zin3_{j}")
                u12 = apool.tile([E, Nc], F32, tag="u12", name=f"u12_{j}")
                stt("dve", u12, k2, -1.0, k1)
                zpart4 = apool.tile([E, Nc], F32, tag="zp4", name=f"zp4_{j}")
                stt("dve", zpart4, u12, hs, z)
                zacc2 = apool.tile([E, Nc], F32, tag="za", name=f"za2_{j}")
                stt("dve", zacc2, k2, 3.0 * hs / 8.0, zacc)

                v2, mg3 = vf_stage(4 * j + 2, zin3)
                zin4, k3 = tail(4 * j + 2, v2, mg3, hs, zpart4, BF16, "zin",
                                f"zin4_{j}")
                zacc3 = apool.tile([E, Nc], F32, tag="za", name=f"za3_{j}")
                stt("dve", zacc3, k3, 3.0 * hs / 8.0, zacc2)

                v2, mg3 = vf_stage(4 * j + 3, zin4)
                if not last:
                    zin, k4 = tail(4 * j + 3, v2, mg3, hs / 8.0, zacc3, BF16,
                                   "zin", f"zin1_{j + 1}")
                    znew = apool.tile([E, Nc], F32, tag="z", name=f"z_{j + 1}")
                    stt("dve", znew, k4, hs / 8.0, zacc3)
                else:
                    znew, _ = tail(4 * j + 3, v2, mg3, hs / 8.0, zacc3, F32,
                                   "zfin", f"z_{j + 1}")
                z = znew

            nc.sync.dma_start(out=d_out[:, :], in_=z)
    nc.finalize()
    return nc


def _b3seg(b3p):
    out = np.zeros((128, 4 * E), np.float32)
    for gi, (d0, nd) in enumerate(DX_GROUPS):
        for i in range(nd):
            out[i, gi * E:(gi + 1) * E] = b3p[(d0 + i) * E:(d0 + i + 1) * E]
    return out


def _sel3(Nc):
    out = np.zeros((128, 3 * Nc), np.float32)
    for i in range(3):
        out[i, i * Nc:(i + 1) * Nc] = 1.0
    return out


def _enable_ldw_opt():
    from concourse import bass_utils as _bu
    if getattr(_bu, "_ldwopt_patched", False):
        return
    _orig = _bu.run_command

    def _run2(argv, **kw):
        argv = ["--enable-ldw-opt=true" if a == "--enable-ldw-opt=false" else a
                for a in argv]
        return _orig(argv, **kw)

    _bu.run_command = _run2
    _bu._ldwopt_patched = True


def _prep_host(t, x, mask, W_embed, b_embed, W0, b0, W1, b1, W2, b2, W3, b3):
    import ml_dtypes
    bf = ml_dtypes.bfloat16

    t = np.asarray(t, np.float32)
    x = np.asarray(x, np.float32)
    mask = np.asarray(mask)
    B, Amax = mask.shape
    N = B * Amax

    C60, h = spline_stage_matrix(t)
    idx = np.flatnonzero(mask.ravel())
    nact = max(1, len(idx))
    Nc = min(512, 4 * ((nact + 4 * N_CORES - 1) // (4 * N_CORES)))
    total = N_CORES * Nc
    pad = np.full(total, idx[0] if len(idx) else 0, dtype=np.int64)
    pad[:len(idx)] = idx
    xp = x.reshape(N, T, D)[pad]

    perm = w3_perm()
    shared = dict(
        b_emb=np.asarray(b_embed, np.float32).reshape(E, 1),
        w0t=np.ascontiguousarray(np.asarray(W0).T).astype(bf),
        w1t=np.ascontiguousarray(np.asarray(W1).T).astype(bf),
        w2t=np.ascontiguousarray(np.asarray(W2).T).astype(bf),
        w3pt=np.ascontiguousarray(np.asarray(W3)[perm].T).astype(bf),
        b012=np.stack([np.asarray(b, np.float32)[m * 128:(m + 1) * 128]
                       for b in (b0, b1, b2) for m in range(4)],
                      axis=1).astype(np.float32),
        b3seg=_b3seg(np.asarray(b3, np.float32)[perm]).astype(bf),
        sel3=_sel3(Nc).astype(bf),
        w_embt=np.concatenate([np.asarray(W_embed, np.float32).T,
                               np.zeros((128 - D, E), np.float32)], 0).astype(bf),
    )
    dx_all = np.einsum("st,ntd->snd", C60, xp.astype(np.float64))  # (60,tot,D)
    in_maps = []
    for c in range(N_CORES):
        xc = xp[c * Nc:(c + 1) * Nc]
        dxc = dx_all[:, c * Nc:(c + 1) * Nc, :]  # (60, Nc, D)
        dxh = np.ascontiguousarray(dxc.transpose(2, 0, 1).reshape(D, -1))
        in_maps.append(dict(
            dxh=dxh.astype(bf),                        # (10, 60*Nc)
            x_t0=np.concatenate([np.ascontiguousarray(xc[:, 0, :].T),
                                 np.zeros((128 - D, Nc), np.float32)],
                                0).astype(bf),
            **shared,
        ))
    return in_maps, pad, len(idx), Nc, h


def kernel(t, x, mask, W_embed, b_embed, W0, b0, W1, b1, W2, b2, W3, b3):
    global last_results
    from concourse import bass_utils
    if os.environ.get("KERNEL_LDWOPT", "0") == "1":
        _enable_ldw_opt()

    mask = np.asarray(mask)
    B, Amax = mask.shape
    N = B * Amax

    in_maps, pad, nact, Nc, h = _prep_host(
        t, x, mask, W_embed, b_embed, W0, b0, W1, b1, W2, b2, W3, b3)

    nc = build_bass2(Nc, h)
    res = bass_utils.run_bass_kernel_spmd(
        nc, in_maps, core_ids=list(range(N_CORES)))
    last_results = res

    zall = np.concatenate([r["zout"].T for r in res.results], 0)  # (total, E)
    out = np.zeros((N, E), np.float32)
    out[pad[:nact]] = zall[:nact]
    return out.reshape(B, Amax, E)


# revision 3
# speedup vs baseline: 1.0504x; 1.0108x over previous
"""Trainium2 Bass kernel for nn_CDE v2: bf16 feature-major pipeline.

Design vs v1:
- Nc = ceil(nact/8) rounded up (132 for the 1044-active case) instead of 256:
  streams scale with actual row count.
- All matmuls bf16 (1 cyc/row at any free width; f32r pays 4x below 256).
- L3 feature-major per-d (W3 stationary, d-major permutation): streams scale
  with Nc, bias folds into the per-partition tanh bias, and the einsum
  reduction moves to one strided DVE tensor_reduce -- no transpose matmuls,
  no bias matmuls.
- dX/dt rows are PE-broadcast into PSUM ([1,128] ones stationary) per stage;
  emitted as next-stage filler behind L3 so the PE stays busy during the
  tanh/mult/reduce tail.
- RK z-update restructured: exactly one fused scalar_tensor_tensor on the
  critical path per sub-stage; all other partials run off-path on Pool.
- PSUM: tag "pp" (mlp m-chunks + L3 per-d, bufs=4) + tag "pdx" (dx broadcast,
  bufs=4) = 8 banks.
"""
import os
import sys
import types

for _p in ("/opt/trn_rl_repo", "/root/.axon_site/_ro/trn_rl_repo"):
    if os.path.isdir(_p) and _p not in sys.path:
        sys.path.insert(0, _p)

if "antenv.axon_hooks" not in sys.modules:
    _m = types.ModuleType("antenv.axon_hooks")
    _hook = [None]

    def _set(hook):
        _hook[0] = hook

    def _get():
        if _hook[0] is None:
            try:
                from trn_agent_boot.trn_boot import _ntff_profile_via_ctypes
                _hook[0] = _ntff_profile_via_ctypes("/opt/axon/libaxon_pjrt.so")
            except Exception:
                pass
        return _hook[0]

    _m.set_axon_ntff_profile_hook = _set
    _m.get_axon_ntff_profile_hook = _get
    sys.modules["antenv.axon_hooks"] = _m

import numpy as np

N_CORES = 8
T, D, E, H = 16, 10, 128, 512
F3 = E * D
N_STEPS = T - 1
N_STAGES = 4 * N_STEPS  # 60
# dx-broadcast d-groups per stage (each tile <= 1 psum bank at Nc<=160)
DX_GROUPS = [(0, 3), (3, 3), (6, 3), (9, 1)]

last_results = None


def spline_stage_matrix(t):
    """C60 (60,16): row 4j+r maps 16 knots of a scalar series to the spline
    derivative at RK stage r of step j.  Also returns h (15,)."""
    t = np.asarray(t, np.float64)
    Tn = len(t)
    h = np.diff(t)
    A = np.zeros((Tn, Tn))
    A[0, 0] = 1.0
    A[-1, -1] = 1.0
    for i in range(1, Tn - 1):
        A[i, i - 1] = h[i - 1]
        A[i, i] = 2.0 * (h[i - 1] + h[i])
        A[i, i + 1] = h[i]
    R = np.zeros((Tn, Tn))
    for i in range(1, Tn - 1):
        R[i, i - 1] = 6.0 / h[i - 1]
        R[i, i] = -6.0 / h[i - 1] - 6.0 / h[i]
        R[i, i + 1] = 6.0 / h[i]
    S = np.linalg.solve(A, R)
    Iden = np.eye(Tn)
    rows = []
    for j in range(Tn - 1):
        hs = h[j]
        for u_frac in (0.0, 1.0 / 3.0, 2.0 / 3.0, 1.0):
            s = t[j + 1] if u_frac == 1.0 else t[j] + u_frac * hs
            i = int(np.clip(np.searchsorted(t, s, side="right") - 1, 0, Tn - 2))
            u = s - t[i]
            b_row = (Iden[i + 1] - Iden[i]) / h[i] - h[i] * (2.0 * S[i] + S[i + 1]) / 6.0
            rows.append(b_row + u * S[i] + (u * u) / (2.0 * h[i]) * (S[i + 1] - S[i]))
    return np.asarray(rows), h


def w3_perm():
    """Permutation so W3p[f'] = W3[e*10+d] with f' = d*128+e (d-major)."""
    fp = np.arange(F3)
    return (fp % E) * D + fp // E


def build_bass2(Nc, h):
    import concourse.bass as bass
    import concourse.bacc as bacc
    import concourse.tile as tile
    import concourse.mybir as mybir

    F32 = mybir.dt.float32
    F32R = mybir.dt.float32r
    BF16 = mybir.dt.bfloat16
    AF = mybir.ActivationFunctionType
    ALU = mybir.AluOpType
    AX = mybir.AxisListType

    nc = bacc.Bacc("TRN2", target_bir_lowering=False)

    d_xt0 = nc.dram_tensor("x_t0", [128, Nc], BF16, kind="ExternalInput")
    d_dxh = nc.dram_tensor("dxh", [D, N_STAGES * Nc], BF16, kind="ExternalInput")
    d_wemb = nc.dram_tensor("w_embt", [128, E], BF16, kind="ExternalInput")
    d_bemb = nc.dram_tensor("b_emb", [E, 1], F32, kind="ExternalInput")
    d_w0 = nc.dram_tensor("w0t", [E, H], BF16, kind="ExternalInput")
    d_w1 = nc.dram_tensor("w1t", [H, H], BF16, kind="ExternalInput")
    d_w2 = nc.dram_tensor("w2t", [H, H], BF16, kind="ExternalInput")
    d_w3 = nc.dram_tensor("w3pt", [H, F3], BF16, kind="ExternalInput")
    d_b012 = nc.dram_tensor("b012", [E, 12], F32, kind="ExternalInput")
    d_b3seg = nc.dram_tensor("b3seg", [128, 4 * E], BF16, kind="ExternalInput")
    d_sel3 = nc.dram_tensor("sel3", [128, 3 * Nc], BF16, kind="ExternalInput")
    d_out = nc.dram_tensor("zout", [E, Nc], F32, kind="ExternalOutput")

    with tile.TileContext(nc) as tc:
        with (
            tc.tile_pool(name="wpool", bufs=1) as wpool,
            tc.tile_pool(name="apool", bufs=2) as apool,
            tc.tile_pool(name="ppool", bufs=4, space="PSUM") as ppool,
        ):
            # ---- weights / constants
            w0t = wpool.tile([E, H], BF16, tag="w0t")
            nc.sync.dma_start(out=w0t, in_=d_w0[:, :])
            w1k = [wpool.tile([128, H], BF16, tag=f"w1k{k}", name=f"w1k{k}")
                   for k in range(4)]
            w2k = [wpool.tile([128, H], BF16, tag=f"w2k{k}", name=f"w2k{k}")
                   for k in range(4)]
            w3k = [wpool.tile([128, F3], BF16, tag=f"w3k{k}", name=f"w3k{k}")
                   for k in range(4)]
            for k in range(4):
                nc.sync.dma_start(out=w1k[k], in_=d_w1[128 * k:128 * (k + 1), :])
                nc.sync.dma_start(out=w2k[k], in_=d_w2[128 * k:128 * (k + 1), :])
                nc.sync.dma_start(out=w3k[k], in_=d_w3[128 * k:128 * (k + 1), :])
            b012 = wpool.tile([E, 12], F32, tag="b012")
            nc.sync.dma_start(out=b012, in_=d_b012[:, :])
            b3seg = wpool.tile([128, 4 * E], BF16, tag="b3seg")
            nc.sync.dma_start(out=b3seg, in_=d_b3seg[:, :])
            sel3 = wpool.tile([128, 3 * Nc], BF16, tag="sel3")
            nc.sync.dma_start(out=sel3, in_=d_sel3[:, :])
            bemb = wpool.tile([E, 1], F32, tag="bemb")
            nc.sync.dma_start(out=bemb, in_=d_bemb[:, :])
            wembt = wpool.tile([128, E], BF16, tag="wembt")
            nc.sync.dma_start(out=wembt, in_=d_wemb[:, :])
            xt0 = wpool.tile([128, Nc], BF16, tag="xt0")
            nc.sync.dma_start(out=xt0, in_=d_xt0[:, :])

            # ---- embed: z0 = W_embed @ x(t0) + b
            pemb = ppool.tile([E, Nc], F32, tag="pp", name="pemb")
            nc.tensor.matmul(pemb, wembt[:, :], xt0[:, :], start=True, stop=True)
            z = apool.tile([E, Nc], F32, tag="z", name="z0")
            nc.scalar.activation(z, pemb, AF.Identity, bias=bemb[:, :], scale=1.0)
            zin = apool.tile([E, Nc], BF16, tag="zin", name="zin0")
            nc.scalar.activation(zin, pemb, AF.Identity, bias=bemb[:, :], scale=1.0)

            # dx rows replicated across partitions via DMA (idle engines)
            dxb_tiles = {}
            l3_tiles = {}

            def emit_l3_bias(s):
                tiles = []
                for gi, (d0, nd) in enumerate(DX_GROUPS):
                    p3 = ppool.tile([128, nd, Nc], F32, tag="pl3", bufs=4,
                                    name=f"p3_{s}_{d0}")
                    nc.tensor.matmul(p3, b3seg[:, gi * E:(gi + 1) * E],
                                     sel3[:, 0:nd * Nc], start=True,
                                     stop=False, skip_group_check=True)
                    tiles.append(p3)
                l3_tiles[s] = tiles

            def emit_bcast(s):
                dxS = wpool.tile([E, D, Nc], BF16, tag="dxS", bufs=3,
                                 name=f"dxS_{s}")
                dap = d_dxh[:, :]
                src_bc = bass.AP(
                    tensor=dap.tensor,
                    offset=s * Nc,
                    ap=[[0, E], [N_STAGES * Nc, D], [1, Nc]])
                nc.sync.dma_start(out=dxS, in_=src_bc)
                dxb_tiles[s] = dxS

            def dxb_ap(s, d):
                return dxb_tiles[s][:, d, :]

            def relu(eng, out_ap, in_ap, bias_ap):
                if eng == "act":
                    nc.scalar.activation(out_ap, in_ap, AF.Relu,
                                         bias=bias_ap, scale=1.0)
                else:
                    e = nc.vector if eng == "dve" else nc.gpsimd
                    e.tensor_scalar(out=out_ap, in0=in_ap,
                                    scalar1=bias_ap, scalar2=0.0,
                                    op0=ALU.add, op1=ALU.max)

            def stt(eng, out_ap, in0_ap, scalar, in1_ap):
                e = nc.vector if eng == "dve" else nc.gpsimd
                e.scalar_tensor_tensor(out=out_ap, in0=in0_ap, scalar=scalar,
                                       in1=in1_ap, op0=ALU.mult, op1=ALU.add)

            RELU_ENG = [["act", "dve", "act", "dve"],
                        ["dve", "act", "dve", "act"],
                        ["act", "dve", "act", "dve"]]
            MULT_ENG = ["pool", "pool", "pool", "pool", "pool",
                        "pool", "pool", "pool", "dve", "dve"]

            emit_bcast(0)
            emit_l3_bias(0)

            def seed_p0(s, in0_ap, in1_ap):
                """p0(s) = W0^T(in0 + in1), two moving passes; the in0 pass
                runs in the previous stage's tail."""
                p0s = []
                for m in range(4):
                    p0 = ppool.tile([128, Nc], F32, tag="pp", name=f"p0_{s}_{m}")
                    nc.tensor.matmul(p0, w0t[:, 128 * m:128 * (m + 1)],
                                     in0_ap, start=True,
                                     stop=(in1_ap is None),
                                     skip_group_check=True)
                    p0s.append(p0)
                if in1_ap is not None:
                    for m in range(4):
                        nc.tensor.matmul(p0s[m], w0t[:, 128 * m:128 * (m + 1)],
                                         in1_ap, start=False, stop=True,
                                         skip_group_check=True)
                return p0s

            def vf_stage(s, p0s):
                """One vector-field eval from pre-seeded L0 psums."""
                y0 = apool.tile([128, 4, Nc], BF16, tag="y0", name=f"y0_{s}")
                for m in range(4):
                    relu(RELU_ENG[0][m], y0[:, m, :], p0s[m], b012[:, m:m + 1])
                # L1
                y1 = apool.tile([128, 4, Nc], BF16, tag="y1", name=f"y1_{s}")
                for m in range(4):
                    p1 = ppool.tile([128, Nc], F32, tag="pp", name=f"p1_{s}_{m}")
                    for k in range(4):
                        nc.tensor.matmul(p1, w1k[k][:, 128 * m:128 * (m + 1)],
                                         y0[:, k, :], start=(k == 0), stop=(k == 3))
                    relu(RELU_ENG[1][m], y1[:, m, :], p1, b012[:, 4 + m:5 + m])
                # L2
                y2 = apool.tile([128, 4, Nc], BF16, tag="y2", name=f"y2_{s}")
                for m in range(4):
                    p2 = ppool.tile([128, Nc], F32, tag="pp", name=f"p2_{s}_{m}")
                    for k in range(4):
                        nc.tensor.matmul(p2, w2k[k][:, 128 * m:128 * (m + 1)],
                                         y1[:, k, :], start=(k == 0), stop=(k == 3))
                    relu(RELU_ENG[2][m], y2[:, m, :], p2, b012[:, 8 + m:9 + m])
                # L3 per 3-d group: bias matmul seeds psum, 4k accumulate,
                # one tanh per group into contiguous y3all
                y3all = apool.tile([128, D, Nc], BF16, tag="y3a", name=f"y3a_{s}")
                mgs = []
                for gi, (d0, nd) in enumerate(DX_GROUPS):
                    p3 = l3_tiles[s][gi]
                    for i in range(nd):
                        d = d0 + i
                        for k in range(4):
                            nc.tensor.matmul(p3[:, i, :],
                                             w3k[k][:, 128 * d:128 * (d + 1)],
                                             y2[:, k, :], start=False,
                                             stop=(k == 3), skip_group_check=True)
                    nc.scalar.activation(y3all[:, d0:d0 + nd, :], p3, AF.Tanh)
                    # einsum partials as soon as each tanh lands (DVE):
                    # mg = y3*dx for this group; running sums off the tail
                    dxS = dxb_tiles[s]
                    mg = apool.tile([128, nd, Nc], BF16, tag=f"mg{gi}",
                                    name=f"mg_{s}_{gi}")
                    nc.vector.tensor_tensor(out=mg, in0=y3all[:, d0:d0 + nd, :],
                                            in1=dxS[:, d0:d0 + nd, :],
                                            op=ALU.mult)
                    mgs.append(mg)
                    if gi == 1:
                        s01 = apool.tile([128, 3, Nc], F32, tag="s01",
                                         name=f"s01_{s}")
                        nc.vector.tensor_tensor(out=s01, in0=mgs[0], in1=mgs[1],
                                                op=ALU.add)
                    elif gi == 2:
                        u3 = apool.tile([128, 3, Nc], F32, tag="u3",
                                        name=f"u3_{s}")
                        nc.vector.tensor_tensor(out=u3, in0=s01, in1=mgs[2],
                                                op=ALU.add)
                        v1 = apool.tile([128, Nc], F32, tag="v1", name=f"v1_{s}")
                        nc.vector.tensor_tensor(out=v1, in0=u3[:, 0, :],
                                                in1=u3[:, 1, :], op=ALU.add)
                        v2 = apool.tile([128, Nc], F32, tag="v2", name=f"v2_{s}")
                        nc.vector.tensor_tensor(out=v2, in0=v1, in1=u3[:, 2, :],
                                                op=ALU.add)
                # PE fillers for the tail: next stage's dx DMA + L3 bias seeds
                if s + 1 < N_STAGES:
                    emit_bcast(s + 1)
                    emit_l3_bias(s + 1)
                return v2, mgs[3][:, 0, :]

            def tail(s, v2, mg3, coef, zbase):
                """Seed p0(s+1) = W0^T(zbase + coef*v2) + W0^T(mg3') where
                mg3' is host-prescaled by coef; k reconstructed off-crit."""
                w = apool.tile([E, Nc], BF16, tag="w", name=f"w_{s}")
                stt("dve", w, v2, coef, zbase)        # off-tail (v2 early)
                p0s = seed_p0(s + 1, w, mg3)
                k_s = apool.tile([E, Nc], F32,
                                 tag=("k1" if s % 4 == 0 else "ks"),
                                 name=f"k_{s}")
                stt("dve", k_s, mg3, 1.0 / coef, v2)  # off-crit
                return p0s, k_s

            p0s = seed_p0(0, zin, None)
            for j in range(N_STEPS):
                hs = float(h[j])
                last = j == N_STEPS - 1

                v2, mg3 = vf_stage(4 * j + 0, p0s)
                p0s, k1 = tail(4 * j, v2, mg3, hs / 3.0, z)
                zpart3 = apool.tile([E, Nc], F32, tag="zp3", name=f"zp3_{j}")
                stt("dve", zpart3, k1, -hs / 3.0, z)
                zacc = apool.tile([E, Nc], F32, tag="za", name=f"za1_{j}")
                stt("dve", zacc, k1, hs / 8.0, z)

                v2, mg3 = vf_stage(4 * j + 1, p0s)
                p0s, k2 = tail(4 * j + 1, v2, mg3, hs, zpart3)
                u12 = apool.tile([E, Nc], F32, tag="u12", name=f"u12_{j}")
                stt("dve", u12, k2, -1.0, k1)
                zpart4 = apool.tile([E, Nc], F32, tag="zp4", name=f"zp4_{j}")
                stt("dve", zpart4, u12, hs, z)
                zacc2 = apool.tile([E, Nc], F32, tag="za", name=f"za2_{j}")
                stt("dve", zacc2, k2, 3.0 * hs / 8.0, zacc)

                v2, mg3 = vf_stage(4 * j + 2, p0s)
                p0s, k3 = tail(4 * j + 2, v2, mg3, hs, zpart4)
                zacc3 = apool.tile([E, Nc], F32, tag="za", name=f"za3_{j}")
                stt("dve", zacc3, k3, 3.0 * hs / 8.0, zacc2)

                v2, mg3 = vf_stage(4 * j + 3, p0s)
                if not last:
                    p0s, k4 = tail(4 * j + 3, v2, mg3, hs / 8.0, zacc3)
                    znew = apool.tile([E, Nc], F32, tag="z", name=f"z_{j + 1}")
                    stt("dve", znew, k4, hs / 8.0, zacc3)
                else:
                    wl = apool.tile([E, Nc], F32, tag="wl", name="wl")
                    stt("dve", wl, v2, hs / 8.0, zacc3)
                    znew = apool.tile([E, Nc], F32, tag="zfin", name=f"z_{j + 1}")
                    nc.vector.tensor_tensor(out=znew, in0=wl, in1=mg3,
                                            op=ALU.add)
                z = znew

            nc.sync.dma_start(out=d_out[:, :], in_=z)
    nc.finalize()
    return nc


def _b3seg(b3p):
    out = np.zeros((128, 4 * E), np.float32)
    for gi, (d0, nd) in enumerate(DX_GROUPS):
        for i in range(nd):
            out[i, gi * E:(gi + 1) * E] = b3p[(d0 + i) * E:(d0 + i + 1) * E]
    return out


def _sel3(Nc):
    out = np.zeros((128, 3 * Nc), np.float32)
    for i in range(3):
        out[i, i * Nc:(i + 1) * Nc] = 1.0
    return out


def _enable_ldw_opt():
    from concourse import bass_utils as _bu
    if getattr(_bu, "_ldwopt_patched", False):
        return
    _orig = _bu.run_command

    def _run2(argv, **kw):
        argv = ["--enable-ldw-opt=true" if a == "--enable-ldw-opt=false" else a
                for a in argv]
        return _orig(argv, **kw)

    _bu.run_command = _run2
    _bu._ldwopt_patched = True


def _prep_host(t, x, mask, W_embed, b_embed, W0, b0, W1, b1, W2, b2, W3, b3):
    import ml_dtypes
    bf = ml_dtypes.bfloat16

    t = np.asarray(t, np.float32)
    x = np.asarray(x, np.float32)
    mask = np.asarray(mask)
    B, Amax = mask.shape
    N = B * Amax

    C60, h = spline_stage_matrix(t)
    idx = np.flatnonzero(mask.ravel())
    nact = max(1, len(idx))
    Nc = min(512, 4 * ((nact + 4 * N_CORES - 1) // (4 * N_CORES)))
    total = N_CORES * Nc
    pad = np.full(total, idx[0] if len(idx) else 0, dtype=np.int64)
    pad[:len(idx)] = idx
    xp = x.reshape(N, T, D)[pad]

    perm = w3_perm()
    shared = dict(
        b_emb=np.asarray(b_embed, np.float32).reshape(E, 1),
        w0t=np.ascontiguousarray(np.asarray(W0).T).astype(bf),
        w1t=np.ascontiguousarray(np.asarray(W1).T).astype(bf),
        w2t=np.ascontiguousarray(np.asarray(W2).T).astype(bf),
        w3pt=np.ascontiguousarray(np.asarray(W3)[perm].T).astype(bf),
        b012=np.stack([np.asarray(b, np.float32)[m * 128:(m + 1) * 128]
                       for b in (b0, b1, b2) for m in range(4)],
                      axis=1).astype(np.float32),
        b3seg=_b3seg(np.asarray(b3, np.float32)[perm]).astype(bf),
        sel3=_sel3(Nc).astype(bf),
        w_embt=np.concatenate([np.asarray(W_embed, np.float32).T,
                               np.zeros((128 - D, E), np.float32)], 0).astype(bf),
    )
    dx_all = np.einsum("st,ntd->snd", C60, xp.astype(np.float64))  # (60,tot,D)
    for s in range(N_STAGES):
        hs = float(h[s // 4])
        coef = (hs / 3.0, hs, hs, hs / 8.0)[s % 4]
        dx_all[s, :, D - 1] *= coef
    in_maps = []
    for c in range(N_CORES):
        xc = xp[c * Nc:(c + 1) * Nc]
        dxc = dx_all[:, c * Nc:(c + 1) * Nc, :]  # (60, Nc, D)
        dxh = np.ascontiguousarray(dxc.transpose(2, 0, 1).reshape(D, -1))
        in_maps.append(dict(
            dxh=dxh.astype(bf),                        # (10, 60*Nc)
            x_t0=np.concatenate([np.ascontiguousarray(xc[:, 0, :].T),
                                 np.zeros((128 - D, Nc), np.float32)],
                                0).astype(bf),
            **shared,
        ))
    return in_maps, pad, len(idx), Nc, h


def kernel(t, x, mask, W_embed, b_embed, W0, b0, W1, b1, W2, b2, W3, b3):
    global last_results
    from concourse import bass_utils
    if os.environ.get("KERNEL_LDWOPT", "0") == "1":
        _enable_ldw_opt()

    mask = np.asarray(mask)
    B, Amax = mask.shape
    N = B * Amax

    in_maps, pad, nact, Nc, h = _prep_host(
        t, x, mask, W_embed, b_embed, W0, b0, W1, b1, W2, b2, W3, b3)

    nc = build_bass2(Nc, h)
    res = bass_utils.run_bass_kernel_spmd(
        nc, in_maps, core_ids=list(range(N_CORES)))
    last_results = res

    zall = np.concatenate([r["zout"].T for r in res.results], 0)  # (total, E)
    out = np.zeros((N, E), np.float32)
    out[pad[:nact]] = zall[:nact]
    return out.reshape(B, Amax, E)


# revision 4
# speedup vs baseline: 1.1431x; 1.0883x over previous
"""Trainium2 Bass kernel for nn_CDE v2: bf16 feature-major pipeline.

Design vs v1:
- Nc = ceil(nact/8) rounded up (132 for the 1044-active case) instead of 256:
  streams scale with actual row count.
- All matmuls bf16 (1 cyc/row at any free width; f32r pays 4x below 256).
- L3 feature-major per-d (W3 stationary, d-major permutation): streams scale
  with Nc, bias folds into the per-partition tanh bias, and the einsum
  reduction moves to one strided DVE tensor_reduce -- no transpose matmuls,
  no bias matmuls.
- dX/dt rows are PE-broadcast into PSUM ([1,128] ones stationary) per stage;
  emitted as next-stage filler behind L3 so the PE stays busy during the
  tanh/mult/reduce tail.
- RK z-update restructured: exactly one fused scalar_tensor_tensor on the
  critical path per sub-stage; all other partials run off-path on Pool.
- PSUM: tag "pp" (mlp m-chunks + L3 per-d, bufs=4) + tag "pdx" (dx broadcast,
  bufs=4) = 8 banks.
"""
import os
import sys
import types

for _p in ("/opt/trn_rl_repo", "/root/.axon_site/_ro/trn_rl_repo"):
    if os.path.isdir(_p) and _p not in sys.path:
        sys.path.insert(0, _p)

if "antenv.axon_hooks" not in sys.modules:
    _m = types.ModuleType("antenv.axon_hooks")
    _hook = [None]

    def _set(hook):
        _hook[0] = hook

    def _get():
        if _hook[0] is None:
            try:
                from trn_agent_boot.trn_boot import _ntff_profile_via_ctypes
                _hook[0] = _ntff_profile_via_ctypes("/opt/axon/libaxon_pjrt.so")
            except Exception:
                pass
        return _hook[0]

    _m.set_axon_ntff_profile_hook = _set
    _m.get_axon_ntff_profile_hook = _get
    sys.modules["antenv.axon_hooks"] = _m

import numpy as np

N_CORES = 8
T, D, E, H = 16, 10, 128, 512
F3 = E * D
N_STEPS = T - 1
N_STAGES = 4 * N_STEPS  # 60
# dx-broadcast d-groups per stage (each tile <= 1 psum bank at Nc<=160)
DX_GROUPS = [(0, 3), (3, 3), (6, 3), (9, 1)]

last_results = None


def spline_stage_matrix(t):
    """C60 (60,16): row 4j+r maps 16 knots of a scalar series to the spline
    derivative at RK stage r of step j.  Also returns h (15,)."""
    t = np.asarray(t, np.float64)
    Tn = len(t)
    h = np.diff(t)
    A = np.zeros((Tn, Tn))
    A[0, 0] = 1.0
    A[-1, -1] = 1.0
    for i in range(1, Tn - 1):
        A[i, i - 1] = h[i - 1]
        A[i, i] = 2.0 * (h[i - 1] + h[i])
        A[i, i + 1] = h[i]
    R = np.zeros((Tn, Tn))
    for i in range(1, Tn - 1):
        R[i, i - 1] = 6.0 / h[i - 1]
        R[i, i] = -6.0 / h[i - 1] - 6.0 / h[i]
        R[i, i + 1] = 6.0 / h[i]
    S = np.linalg.solve(A, R)
    Iden = np.eye(Tn)
    rows = []
    for j in range(Tn - 1):
        hs = h[j]
        for u_frac in (0.0, 1.0 / 3.0, 2.0 / 3.0, 1.0):
            s = t[j + 1] if u_frac == 1.0 else t[j] + u_frac * hs
            i = int(np.clip(np.searchsorted(t, s, side="right") - 1, 0, Tn - 2))
            u = s - t[i]
            b_row = (Iden[i + 1] - Iden[i]) / h[i] - h[i] * (2.0 * S[i] + S[i + 1]) / 6.0
            rows.append(b_row + u * S[i] + (u * u) / (2.0 * h[i]) * (S[i + 1] - S[i]))
    return np.asarray(rows), h


def w3_perm():
    """Permutation so W3p[f'] = W3[e*10+d] with f' = d*128+e (d-major)."""
    fp = np.arange(F3)
    return (fp % E) * D + fp // E


def build_bass2(Nc, h):
    import concourse.bass as bass
    import concourse.bacc as bacc
    import concourse.tile as tile
    import concourse.mybir as mybir

    F32 = mybir.dt.float32
    F32R = mybir.dt.float32r
    BF16 = mybir.dt.bfloat16
    AF = mybir.ActivationFunctionType
    ALU = mybir.AluOpType
    AX = mybir.AxisListType

    nc = bacc.Bacc("TRN2", target_bir_lowering=False)

    d_xt0 = nc.dram_tensor("x_t0", [128, Nc], BF16, kind="ExternalInput")
    d_dxh = nc.dram_tensor("dxh", [D, N_STAGES * Nc], BF16, kind="ExternalInput")
    d_wemb = nc.dram_tensor("w_embt", [128, E], BF16, kind="ExternalInput")
    d_bemb = nc.dram_tensor("b_emb", [E, 1], F32, kind="ExternalInput")
    d_w0 = nc.dram_tensor("w0t", [E, H], BF16, kind="ExternalInput")
    d_w1 = nc.dram_tensor("w1t", [H, H], BF16, kind="ExternalInput")
    d_w2 = nc.dram_tensor("w2t", [H, H], BF16, kind="ExternalInput")
    d_w3 = nc.dram_tensor("w3pt", [H, F3], BF16, kind="ExternalInput")
    d_b012 = nc.dram_tensor("b012", [E, 12], F32, kind="ExternalInput")
    d_b3seg = nc.dram_tensor("b3seg", [128, 4 * E], BF16, kind="ExternalInput")
    d_sel3 = nc.dram_tensor("sel3", [128, 3 * Nc], BF16, kind="ExternalInput")
    d_out = nc.dram_tensor("zout", [E, Nc], F32, kind="ExternalOutput")

    with tile.TileContext(nc) as tc:
        with (
            tc.tile_pool(name="wpool", bufs=1) as wpool,
            tc.tile_pool(name="apool", bufs=2) as apool,
            tc.tile_pool(name="ppool", bufs=5, space="PSUM") as ppool,
        ):
            # ---- weights / constants
            w0t = wpool.tile([E, H], BF16, tag="w0t")
            nc.sync.dma_start(out=w0t, in_=d_w0[:, :])
            w1k = [wpool.tile([128, H], BF16, tag=f"w1k{k}", name=f"w1k{k}")
                   for k in range(4)]
            w2k = [wpool.tile([128, H], BF16, tag=f"w2k{k}", name=f"w2k{k}")
                   for k in range(4)]
            w3k = [wpool.tile([128, F3], BF16, tag=f"w3k{k}", name=f"w3k{k}")
                   for k in range(4)]
            for k in range(4):
                nc.sync.dma_start(out=w1k[k], in_=d_w1[128 * k:128 * (k + 1), :])
                nc.sync.dma_start(out=w2k[k], in_=d_w2[128 * k:128 * (k + 1), :])
                nc.sync.dma_start(out=w3k[k], in_=d_w3[128 * k:128 * (k + 1), :])
            b012 = wpool.tile([E, 12], F32, tag="b012")
            nc.sync.dma_start(out=b012, in_=d_b012[:, :])
            b3seg = wpool.tile([128, 4 * E], BF16, tag="b3seg")
            nc.sync.dma_start(out=b3seg, in_=d_b3seg[:, :])
            sel3 = wpool.tile([128, 3 * Nc], BF16, tag="sel3")
            nc.sync.dma_start(out=sel3, in_=d_sel3[:, :])
            bemb = wpool.tile([E, 1], F32, tag="bemb")
            nc.sync.dma_start(out=bemb, in_=d_bemb[:, :])
            wembt = wpool.tile([128, E], BF16, tag="wembt")
            nc.sync.dma_start(out=wembt, in_=d_wemb[:, :])
            xt0 = wpool.tile([128, Nc], BF16, tag="xt0")
            nc.sync.dma_start(out=xt0, in_=d_xt0[:, :])

            # ---- embed: z0 = W_embed @ x(t0) + b
            pemb = ppool.tile([E, Nc], F32, tag="pp", name="pemb")
            nc.tensor.matmul(pemb, wembt[:, :], xt0[:, :], start=True, stop=True)
            z = apool.tile([E, Nc], F32, tag="z", name="z0")
            nc.scalar.activation(z, pemb, AF.Identity, bias=bemb[:, :], scale=1.0)
            zin = apool.tile([E, Nc], BF16, tag="zin", name="zin0")
            nc.scalar.activation(zin, pemb, AF.Identity, bias=bemb[:, :], scale=1.0)

            # dx rows replicated across partitions via DMA (idle engines)
            dxb_tiles = {}
            l3_tiles = {}

            def emit_l3_bias(s):
                tiles = []
                for gi, (d0, nd) in enumerate(DX_GROUPS):
                    p3 = ppool.tile([128, nd, Nc], F32, tag="pl3", bufs=3,
                                    name=f"p3_{s}_{d0}")
                    nc.tensor.matmul(p3, b3seg[:, gi * E:(gi + 1) * E],
                                     sel3[:, 0:nd * Nc], start=True,
                                     stop=False, skip_group_check=True)
                    tiles.append(p3)
                l3_tiles[s] = tiles

            def emit_bcast(s):
                dxS = wpool.tile([E, D, Nc], BF16, tag="dxS", bufs=3,
                                 name=f"dxS_{s}")
                dap = d_dxh[:, :]
                src_bc = bass.AP(
                    tensor=dap.tensor,
                    offset=s * Nc,
                    ap=[[0, E], [N_STAGES * Nc, D], [1, Nc]])
                nc.sync.dma_start(out=dxS, in_=src_bc)
                dxb_tiles[s] = dxS

            def dxb_ap(s, d):
                return dxb_tiles[s][:, d, :]

            def relu(eng, out_ap, in_ap, bias_ap):
                if eng == "act":
                    nc.scalar.activation(out_ap, in_ap, AF.Relu,
                                         bias=bias_ap, scale=1.0)
                else:
                    e = nc.vector if eng == "dve" else nc.gpsimd
                    e.tensor_scalar(out=out_ap, in0=in_ap,
                                    scalar1=bias_ap, scalar2=0.0,
                                    op0=ALU.add, op1=ALU.max)

            def stt(eng, out_ap, in0_ap, scalar, in1_ap):
                e = nc.vector if eng == "dve" else nc.gpsimd
                e.scalar_tensor_tensor(out=out_ap, in0=in0_ap, scalar=scalar,
                                       in1=in1_ap, op0=ALU.mult, op1=ALU.add)

            RELU_ENG = [["act", "dve", "act", "dve"],
                        ["dve", "act", "dve", "act"],
                        ["act", "dve", "act", "dve"]]
            MULT_ENG = ["pool", "pool", "pool", "pool", "pool",
                        "pool", "pool", "pool", "dve", "dve"]

            emit_bcast(0)
            emit_l3_bias(0)

            def seed_p0(s, in0_ap, in1_ap):
                """p0(s) = W0^T(in0 + in1), two moving passes; the in0 pass
                runs in the previous stage's tail."""
                p0s = []
                for m in range(4):
                    p0 = ppool.tile([128, Nc], F32, tag="pp", name=f"p0_{s}_{m}")
                    nc.tensor.matmul(p0, w0t[:, 128 * m:128 * (m + 1)],
                                     in0_ap, start=True,
                                     stop=(in1_ap is None),
                                     skip_group_check=True)
                    p0s.append(p0)
                if in1_ap is not None:
                    for m in range(4):
                        nc.tensor.matmul(p0s[m], w0t[:, 128 * m:128 * (m + 1)],
                                         in1_ap, start=False, stop=True,
                                         skip_group_check=True)
                return p0s

            def vf_stage(s, p0s):
                """One vector-field eval from pre-seeded L0 psums."""
                y0 = apool.tile([128, 4, Nc], BF16, tag="y0", name=f"y0_{s}")
                for m in range(4):
                    relu(RELU_ENG[0][m], y0[:, m, :], p0s[m], b012[:, m:m + 1])
                # L1
                y1 = apool.tile([128, 4, Nc], BF16, tag="y1", name=f"y1_{s}")
                for m in range(4):
                    p1 = ppool.tile([128, Nc], F32, tag="pp", name=f"p1_{s}_{m}")
                    for k in range(4):
                        nc.tensor.matmul(p1, w1k[k][:, 128 * m:128 * (m + 1)],
                                         y0[:, k, :], start=(k == 0), stop=(k == 3))
                    relu(RELU_ENG[1][m], y1[:, m, :], p1, b012[:, 4 + m:5 + m])
                # L2
                y2 = apool.tile([128, 4, Nc], BF16, tag="y2", name=f"y2_{s}")
                for m in range(4):
                    p2 = ppool.tile([128, Nc], F32, tag="pp", name=f"p2_{s}_{m}")
                    for k in range(4):
                        nc.tensor.matmul(p2, w2k[k][:, 128 * m:128 * (m + 1)],
                                         y1[:, k, :], start=(k == 0), stop=(k == 3))
                    relu(RELU_ENG[2][m], y2[:, m, :], p2, b012[:, 8 + m:9 + m])
                # L3 per 3-d group: bias matmul seeds psum, 4k accumulate,
                # one tanh per group into contiguous y3all
                y3all = apool.tile([128, D, Nc], BF16, tag="y3a", name=f"y3a_{s}")
                mgs = []
                for gi, (d0, nd) in enumerate(DX_GROUPS):
                    p3 = l3_tiles[s][gi]
                    for i in range(nd):
                        d = d0 + i
                        for k in range(4):
                            nc.tensor.matmul(p3[:, i, :],
                                             w3k[k][:, 128 * d:128 * (d + 1)],
                                             y2[:, k, :], start=False,
                                             stop=(k == 3), skip_group_check=True)
                    nc.scalar.activation(y3all[:, d0:d0 + nd, :], p3, AF.Tanh)
                    # einsum partials as soon as each tanh lands (DVE):
                    # mg = y3*dx for this group; running sums off the tail
                    dxS = dxb_tiles[s]
                    mg = apool.tile([128, nd, Nc], BF16, tag=f"mg{gi}",
                                    name=f"mg_{s}_{gi}")
                    nc.vector.tensor_tensor(out=mg, in0=y3all[:, d0:d0 + nd, :],
                                            in1=dxS[:, d0:d0 + nd, :],
                                            op=ALU.mult)
                    mgs.append(mg)
                    if gi == 1:
                        s01 = apool.tile([128, 3, Nc], F32, tag="s01",
                                         name=f"s01_{s}")
                        nc.vector.tensor_tensor(out=s01, in0=mgs[0], in1=mgs[1],
                                                op=ALU.add)
                    elif gi == 2:
                        u3 = apool.tile([128, 3, Nc], F32, tag="u3",
                                        name=f"u3_{s}")
                        nc.vector.tensor_tensor(out=u3, in0=s01, in1=mgs[2],
                                                op=ALU.add)
                        v1 = apool.tile([128, Nc], F32, tag="v1", name=f"v1_{s}")
                        nc.vector.tensor_tensor(out=v1, in0=u3[:, 0, :],
                                                in1=u3[:, 1, :], op=ALU.add)
                        v2 = apool.tile([128, Nc], F32, tag="v2", name=f"v2_{s}")
                        nc.vector.tensor_tensor(out=v2, in0=v1, in1=u3[:, 2, :],
                                                op=ALU.add)
                # PE fillers for the tail: next stage's dx DMA + L3 bias seeds
                if s + 1 < N_STAGES:
                    emit_bcast(s + 1)
                    emit_l3_bias(s + 1)
                return v2, mgs[3][:, 0, :]

            def tail(s, v2, mg3, coef, zbase):
                """Seed p0(s+1) = W0^T(zbase + coef*v2) + W0^T(mg3') where
                mg3' is host-prescaled by coef; k reconstructed off-crit."""
                w = apool.tile([E, Nc], BF16, tag="w", name=f"w_{s}")
                stt("dve", w, v2, coef, zbase)        # off-tail (v2 early)
                p0s = seed_p0(s + 1, w, mg3)
                k_s = apool.tile([E, Nc], F32,
                                 tag=("k1" if s % 4 == 0 else "ks"),
                                 name=f"k_{s}")
                stt("dve", k_s, mg3, 1.0 / coef, v2)  # off-crit
                return p0s, k_s

            p0s = seed_p0(0, zin, None)
            for j in range(N_STEPS):
                hs = float(h[j])
                last = j == N_STEPS - 1

                v2, mg3 = vf_stage(4 * j + 0, p0s)
                p0s, k1 = tail(4 * j, v2, mg3, hs / 3.0, z)
                zpart3 = apool.tile([E, Nc], F32, tag="zp3", name=f"zp3_{j}")
                stt("dve", zpart3, k1, -hs / 3.0, z)
                zacc = apool.tile([E, Nc], F32, tag="za", name=f"za1_{j}")
                stt("dve", zacc, k1, hs / 8.0, z)

                v2, mg3 = vf_stage(4 * j + 1, p0s)
                p0s, k2 = tail(4 * j + 1, v2, mg3, hs, zpart3)
                u12 = apool.tile([E, Nc], F32, tag="u12", name=f"u12_{j}")
                stt("dve", u12, k2, -1.0, k1)
                zpart4 = apool.tile([E, Nc], F32, tag="zp4", name=f"zp4_{j}")
                stt("dve", zpart4, u12, hs, z)
                zacc2 = apool.tile([E, Nc], F32, tag="za", name=f"za2_{j}")
                stt("dve", zacc2, k2, 3.0 * hs / 8.0, zacc)

                v2, mg3 = vf_stage(4 * j + 2, p0s)
                p0s, k3 = tail(4 * j + 2, v2, mg3, hs, zpart4)
                zacc3 = apool.tile([E, Nc], F32, tag="za", name=f"za3_{j}")
                stt("dve", zacc3, k3, 3.0 * hs / 8.0, zacc2)

                v2, mg3 = vf_stage(4 * j + 3, p0s)
                if not last:
                    p0s, k4 = tail(4 * j + 3, v2, mg3, hs / 8.0, zacc3)
                    znew = apool.tile([E, Nc], F32, tag="z", name=f"z_{j + 1}")
                    stt("dve", znew, k4, hs / 8.0, zacc3)
                else:
                    wl = apool.tile([E, Nc], F32, tag="wl", name="wl")
                    stt("dve", wl, v2, hs / 8.0, zacc3)
                    znew = apool.tile([E, Nc], F32, tag="zfin", name=f"z_{j + 1}")
                    nc.vector.tensor_tensor(out=znew, in0=wl, in1=mg3,
                                            op=ALU.add)
                z = znew

            nc.sync.dma_start(out=d_out[:, :], in_=z)
    nc.finalize()
    return nc


def _b3seg(b3p):
    out = np.zeros((128, 4 * E), np.float32)
    for gi, (d0, nd) in enumerate(DX_GROUPS):
        for i in range(nd):
            out[i, gi * E:(gi + 1) * E] = b3p[(d0 + i) * E:(d0 + i + 1) * E]
    return out


def _sel3(Nc):
    out = np.zeros((128, 3 * Nc), np.float32)
    for i in range(3):
        out[i, i * Nc:(i + 1) * Nc] = 1.0
    return out


def _enable_ldw_opt():
    from concourse import bass_utils as _bu
    if getattr(_bu, "_ldwopt_patched", False):
        return
    _orig = _bu.run_command

    def _run2(argv, **kw):
        argv = ["--enable-ldw-opt=true" if a == "--enable-ldw-opt=false" else a
                for a in argv]
        return _orig(argv, **kw)

    _bu.run_command = _run2
    _bu._ldwopt_patched = True


def _prep_host(t, x, mask, W_embed, b_embed, W0, b0, W1, b1, W2, b2, W3, b3):
    import ml_dtypes
    bf = ml_dtypes.bfloat16

    t = np.asarray(t, np.float32)
    x = np.asarray(x, np.float32)
    mask = np.asarray(mask)
    B, Amax = mask.shape
    N = B * Amax

    C60, h = spline_stage_matrix(t)
    idx = np.flatnonzero(mask.ravel())
    nact = max(1, len(idx))
    Nc = min(512, 4 * ((nact + 4 * N_CORES - 1) // (4 * N_CORES)))
    total = N_CORES * Nc
    pad = np.full(total, idx[0] if len(idx) else 0, dtype=np.int64)
    pad[:len(idx)] = idx
    xp = x.reshape(N, T, D)[pad]

    perm = w3_perm()
    shared = dict(
        b_emb=np.asarray(b_embed, np.float32).reshape(E, 1),
        w0t=np.ascontiguousarray(np.asarray(W0).T).astype(bf),
        w1t=np.ascontiguousarray(np.asarray(W1).T).astype(bf),
        w2t=np.ascontiguousarray(np.asarray(W2).T).astype(bf),
        w3pt=np.ascontiguousarray(np.asarray(W3)[perm].T).astype(bf),
        b012=np.stack([np.asarray(b, np.float32)[m * 128:(m + 1) * 128]
                       for b in (b0, b1, b2) for m in range(4)],
                      axis=1).astype(np.float32),
        b3seg=_b3seg(np.asarray(b3, np.float32)[perm]).astype(bf),
        sel3=_sel3(Nc).astype(bf),
        w_embt=np.concatenate([np.asarray(W_embed, np.float32).T,
                               np.zeros((128 - D, E), np.float32)], 0).astype(bf),
    )
    dx_all = np.einsum("st,ntd->snd", C60, xp.astype(np.float64))  # (60,tot,D)
    for s in range(N_STAGES):
        hs = float(h[s // 4])
        coef = (hs / 3.0, hs, hs, hs / 8.0)[s % 4]
        dx_all[s, :, D - 1] *= coef
    in_maps = []
    for c in range(N_CORES):
        xc = xp[c * Nc:(c + 1) * Nc]
        dxc = dx_all[:, c * Nc:(c + 1) * Nc, :]  # (60, Nc, D)
        dxh = np.ascontiguousarray(dxc.transpose(2, 0, 1).reshape(D, -1))
        in_maps.append(dict(
            dxh=dxh.astype(bf),                        # (10, 60*Nc)
            x_t0=np.concatenate([np.ascontiguousarray(xc[:, 0, :].T),
                                 np.zeros((128 - D, Nc), np.float32)],
                                0).astype(bf),
            **shared,
        ))
    return in_maps, pad, len(idx), Nc, h


def kernel(t, x, mask, W_embed, b_embed, W0, b0, W1, b1, W2, b2, W3, b3):
    global last_results
    from concourse import bass_utils
    if os.environ.get("KERNEL_LDWOPT", "0") == "1":
        _enable_ldw_opt()

    mask = np.asarray(mask)
    B, Amax = mask.shape
    N = B * Amax

    in_maps, pad, nact, Nc, h = _prep_host(
        t, x, mask, W_embed, b_embed, W0, b0, W1, b1, W2, b2, W3, b3)

    nc = build_bass2(Nc, h)
    res = bass_utils.run_bass_kernel_spmd(
        nc, in_maps, core_ids=list(range(N_CORES)))
    last_results = res

    zall = np.concatenate([r["zout"].T for r in res.results], 0)  # (total, E)
    out = np.zeros((N, E), np.float32)
    out[pad[:nact]] = zall[:nact]
    return out.reshape(B, Amax, E)


# revision 5
# speedup vs baseline: 1.1481x; 1.0044x over previous
"""Trainium2 Bass kernel for nn_CDE v2: bf16 feature-major pipeline.

Design vs v1:
- Nc = ceil(nact/8) rounded up (132 for the 1044-active case) instead of 256:
  streams scale with actual row count.
- All matmuls bf16 (1 cyc/row at any free width; f32r pays 4x below 256).
- L3 feature-major per-d (W3 stationary, d-major permutation): streams scale
  with Nc, bias folds into the per-partition tanh bias, and the einsum
  reduction moves to one strided DVE tensor_reduce -- no transpose matmuls,
  no bias matmuls.
- dX/dt rows are PE-broadcast into PSUM ([1,128] ones stationary) per stage;
  emitted as next-stage filler behind L3 so the PE stays busy during the
  tanh/mult/reduce tail.
- RK z-update restructured: exactly one fused scalar_tensor_tensor on the
  critical path per sub-stage; all other partials run off-path on Pool.
- PSUM: tag "pp" (mlp m-chunks + L3 per-d, bufs=4) + tag "pdx" (dx broadcast,
  bufs=4) = 8 banks.
"""
import os
import sys
import types

for _p in ("/opt/trn_rl_repo", "/root/.axon_site/_ro/trn_rl_repo"):
    if os.path.isdir(_p) and _p not in sys.path:
        sys.path.insert(0, _p)

if "antenv.axon_hooks" not in sys.modules:
    _m = types.ModuleType("antenv.axon_hooks")
    _hook = [None]

    def _set(hook):
        _hook[0] = hook

    def _get():
        if _hook[0] is None:
            try:
                from trn_agent_boot.trn_boot import _ntff_profile_via_ctypes
                _hook[0] = _ntff_profile_via_ctypes("/opt/axon/libaxon_pjrt.so")
            except Exception:
                pass
        return _hook[0]

    _m.set_axon_ntff_profile_hook = _set
    _m.get_axon_ntff_profile_hook = _get
    sys.modules["antenv.axon_hooks"] = _m

import numpy as np

N_CORES = 8
T, D, E, H = 16, 10, 128, 512
F3 = E * D
N_STEPS = T - 1
N_STAGES = 4 * N_STEPS  # 60
# dx-broadcast d-groups per stage (each tile <= 1 psum bank at Nc<=160)
DX_GROUPS = [(0, 3), (3, 3), (6, 3), (9, 1)]

last_results = None


def spline_stage_matrix(t):
    """C60 (60,16): row 4j+r maps 16 knots of a scalar series to the spline
    derivative at RK stage r of step j.  Also returns h (15,)."""
    t = np.asarray(t, np.float64)
    Tn = len(t)
    h = np.diff(t)
    A = np.zeros((Tn, Tn))
    A[0, 0] = 1.0
    A[-1, -1] = 1.0
    for i in range(1, Tn - 1):
        A[i, i - 1] = h[i - 1]
        A[i, i] = 2.0 * (h[i - 1] + h[i])
        A[i, i + 1] = h[i]
    R = np.zeros((Tn, Tn))
    for i in range(1, Tn - 1):
        R[i, i - 1] = 6.0 / h[i - 1]
        R[i, i] = -6.0 / h[i - 1] - 6.0 / h[i]
        R[i, i + 1] = 6.0 / h[i]
    S = np.linalg.solve(A, R)
    Iden = np.eye(Tn)
    rows = []
    for j in range(Tn - 1):
        hs = h[j]
        for u_frac in (0.0, 1.0 / 3.0, 2.0 / 3.0, 1.0):
            s = t[j + 1] if u_frac == 1.0 else t[j] + u_frac * hs
            i = int(np.clip(np.searchsorted(t, s, side="right") - 1, 0, Tn - 2))
            u = s - t[i]
            b_row = (Iden[i + 1] - Iden[i]) / h[i] - h[i] * (2.0 * S[i] + S[i + 1]) / 6.0
            rows.append(b_row + u * S[i] + (u * u) / (2.0 * h[i]) * (S[i + 1] - S[i]))
    return np.asarray(rows), h


def w3_perm():
    """Permutation so W3p[f'] = W3[e*10+d] with f' = d*128+e (d-major)."""
    fp = np.arange(F3)
    return (fp % E) * D + fp // E


def build_bass2(Nc, h):
    import concourse.bass as bass
    import concourse.bacc as bacc
    import concourse.tile as tile
    import concourse.mybir as mybir

    F32 = mybir.dt.float32
    F32R = mybir.dt.float32r
    BF16 = mybir.dt.bfloat16
    AF = mybir.ActivationFunctionType
    ALU = mybir.AluOpType
    AX = mybir.AxisListType

    nc = bacc.Bacc("TRN2", target_bir_lowering=False)

    d_xt0 = nc.dram_tensor("x_t0", [128, Nc], BF16, kind="ExternalInput")
    d_dxh = nc.dram_tensor("dxh", [D, N_STAGES * Nc], BF16, kind="ExternalInput")
    d_wemb = nc.dram_tensor("w_embt", [128, E], BF16, kind="ExternalInput")
    d_bemb = nc.dram_tensor("b_emb", [E, 1], F32, kind="ExternalInput")
    d_w0 = nc.dram_tensor("w0t", [E, H], BF16, kind="ExternalInput")
    d_w1 = nc.dram_tensor("w1t", [H, H], BF16, kind="ExternalInput")
    d_w2 = nc.dram_tensor("w2t", [H, H], BF16, kind="ExternalInput")
    d_w3 = nc.dram_tensor("w3pt", [H, F3], BF16, kind="ExternalInput")
    d_b012 = nc.dram_tensor("b012", [E, 12], F32, kind="ExternalInput")
    d_b3seg = nc.dram_tensor("b3seg", [128, 4 * E], BF16, kind="ExternalInput")
    d_sel3 = nc.dram_tensor("sel3", [128, 3 * Nc], BF16, kind="ExternalInput")
    d_out = nc.dram_tensor("zout", [E, Nc], F32, kind="ExternalOutput")

    with tile.TileContext(nc) as tc:
        with (
            tc.tile_pool(name="wpool", bufs=1) as wpool,
            tc.tile_pool(name="apool", bufs=2) as apool,
            tc.tile_pool(name="ppool", bufs=5, space="PSUM") as ppool,
        ):
            # ---- weights / constants
            w0t = wpool.tile([E, H], BF16, tag="w0t")
            nc.sync.dma_start(out=w0t, in_=d_w0[:, :])
            w1k = [wpool.tile([128, H], BF16, tag=f"w1k{k}", name=f"w1k{k}")
                   for k in range(4)]
            w2k = [wpool.tile([128, H], BF16, tag=f"w2k{k}", name=f"w2k{k}")
                   for k in range(4)]
            w3k = [wpool.tile([128, F3], BF16, tag=f"w3k{k}", name=f"w3k{k}")
                   for k in range(4)]
            for k in range(4):
                nc.sync.dma_start(out=w1k[k], in_=d_w1[128 * k:128 * (k + 1), :])
                nc.sync.dma_start(out=w2k[k], in_=d_w2[128 * k:128 * (k + 1), :])
                nc.sync.dma_start(out=w3k[k], in_=d_w3[128 * k:128 * (k + 1), :])
            b012 = wpool.tile([E, 12], F32, tag="b012")
            nc.sync.dma_start(out=b012, in_=d_b012[:, :])
            b3seg = wpool.tile([128, 4 * E], BF16, tag="b3seg")
            nc.sync.dma_start(out=b3seg, in_=d_b3seg[:, :])
            sel3 = wpool.tile([128, 3 * Nc], BF16, tag="sel3")
            nc.sync.dma_start(out=sel3, in_=d_sel3[:, :])
            bemb = wpool.tile([E, 1], F32, tag="bemb")
            nc.sync.dma_start(out=bemb, in_=d_bemb[:, :])
            wembt = wpool.tile([128, E], BF16, tag="wembt")
            nc.sync.dma_start(out=wembt, in_=d_wemb[:, :])
            xt0 = wpool.tile([128, Nc], BF16, tag="xt0")
            nc.sync.dma_start(out=xt0, in_=d_xt0[:, :])

            # ---- embed: z0 = W_embed @ x(t0) + b
            pemb = ppool.tile([E, Nc], F32, tag="pp", name="pemb")
            nc.tensor.matmul(pemb, wembt[:, :], xt0[:, :], start=True, stop=True)
            z = apool.tile([E, Nc], F32, tag="z", name="z0")
            nc.scalar.activation(z, pemb, AF.Identity, bias=bemb[:, :], scale=1.0)
            zin = apool.tile([E, Nc], BF16, tag="zin", name="zin0")
            nc.scalar.activation(zin, pemb, AF.Identity, bias=bemb[:, :], scale=1.0)

            # dx rows replicated across partitions via DMA (idle engines)
            dxb_tiles = {}
            l3_tiles = {}

            def emit_l3_bias(s):
                tiles = []
                for gi, (d0, nd) in enumerate(DX_GROUPS):
                    p3 = ppool.tile([128, nd, Nc], F32, tag="pl3", bufs=3,
                                    name=f"p3_{s}_{d0}")
                    nc.tensor.matmul(p3, b3seg[:, gi * E:(gi + 1) * E],
                                     sel3[:, 0:nd * Nc], start=True,
                                     stop=False, skip_group_check=True)
                    tiles.append(p3)
                l3_tiles[s] = tiles

            def emit_bcast(s):
                dxS = wpool.tile([E, D, Nc], BF16, tag="dxS", bufs=3,
                                 name=f"dxS_{s}")
                dap = d_dxh[:, :]
                src_bc = bass.AP(
                    tensor=dap.tensor,
                    offset=s * Nc,
                    ap=[[0, E], [N_STAGES * Nc, D], [1, Nc]])
                nc.sync.dma_start(out=dxS, in_=src_bc)
                dxb_tiles[s] = dxS

            def dxb_ap(s, d):
                return dxb_tiles[s][:, d, :]

            def relu(eng, out_ap, in_ap, bias_ap):
                if eng == "act":
                    nc.scalar.activation(out_ap, in_ap, AF.Relu,
                                         bias=bias_ap, scale=1.0)
                else:
                    e = nc.vector if eng == "dve" else nc.gpsimd
                    e.tensor_scalar(out=out_ap, in0=in_ap,
                                    scalar1=bias_ap, scalar2=0.0,
                                    op0=ALU.add, op1=ALU.max)

            def stt(eng, out_ap, in0_ap, scalar, in1_ap):
                e = nc.vector if eng == "dve" else nc.gpsimd
                e.scalar_tensor_tensor(out=out_ap, in0=in0_ap, scalar=scalar,
                                       in1=in1_ap, op0=ALU.mult, op1=ALU.add)

            RELU_ENG = [["act", "dve", "act", "dve"],
                        ["dve", "act", "dve", "act"],
                        ["act", "dve", "act", "dve"]]
            MULT_ENG = ["pool", "pool", "pool", "pool", "pool",
                        "pool", "pool", "pool", "dve", "dve"]

            emit_bcast(0)
            emit_l3_bias(0)

            def seed_p0(s, in0_ap, in1_ap):
                """p0(s) = W0^T(in0 + in1), two moving passes; the in0 pass
                runs in the previous stage's tail."""
                p0s = []
                first = in1_ap if in1_ap is not None else in0_ap
                for m in range(4):
                    p0 = ppool.tile([128, Nc], F32, tag="pp", name=f"p0_{s}_{m}")
                    nc.tensor.matmul(p0, w0t[:, 128 * m:128 * (m + 1)],
                                     first, start=True,
                                     stop=(in1_ap is None),
                                     skip_group_check=True)
                    p0s.append(p0)
                if in1_ap is not None:
                    for m in range(4):
                        nc.tensor.matmul(p0s[m], w0t[:, 128 * m:128 * (m + 1)],
                                         in0_ap, start=False, stop=True,
                                         skip_group_check=True)
                return p0s

            def vf_stage(s, p0s):
                """One vector-field eval from pre-seeded L0 psums."""
                y0 = apool.tile([128, 4, Nc], BF16, tag="y0", name=f"y0_{s}")
                for m in range(4):
                    relu(RELU_ENG[0][m], y0[:, m, :], p0s[m], b012[:, m:m + 1])
                # L1
                y1 = apool.tile([128, 4, Nc], BF16, tag="y1", name=f"y1_{s}")
                for m in range(4):
                    p1 = ppool.tile([128, Nc], F32, tag="pp", name=f"p1_{s}_{m}")
                    for k in range(4):
                        nc.tensor.matmul(p1, w1k[k][:, 128 * m:128 * (m + 1)],
                                         y0[:, k, :], start=(k == 0), stop=(k == 3))
                    relu(RELU_ENG[1][m], y1[:, m, :], p1, b012[:, 4 + m:5 + m])
                # L2
                y2 = apool.tile([128, 4, Nc], BF16, tag="y2", name=f"y2_{s}")
                for m in range(4):
                    p2 = ppool.tile([128, Nc], F32, tag="pp", name=f"p2_{s}_{m}")
                    for k in range(4):
                        nc.tensor.matmul(p2, w2k[k][:, 128 * m:128 * (m + 1)],
                                         y1[:, k, :], start=(k == 0), stop=(k == 3))
                    relu(RELU_ENG[2][m], y2[:, m, :], p2, b012[:, 8 + m:9 + m])
                # L3 per 3-d group: bias matmul seeds psum, 4k accumulate,
                # one tanh per group into contiguous y3all
                y3all = apool.tile([128, D, Nc], BF16, tag="y3a", name=f"y3a_{s}")
                mgs = []
                for gi, (d0, nd) in enumerate(DX_GROUPS):
                    p3 = l3_tiles[s][gi]
                    for i in range(nd):
                        d = d0 + i
                        for k in range(4):
                            nc.tensor.matmul(p3[:, i, :],
                                             w3k[k][:, 128 * d:128 * (d + 1)],
                                             y2[:, k, :], start=False,
                                             stop=(k == 3), skip_group_check=True)
                    nc.scalar.activation(y3all[:, d0:d0 + nd, :], p3, AF.Tanh)
                    # einsum partials as soon as each tanh lands (DVE):
                    # mg = y3*dx for this group; running sums off the tail
                    dxS = dxb_tiles[s]
                    mg = apool.tile([128, nd, Nc], BF16, tag=f"mg{gi}",
                                    name=f"mg_{s}_{gi}")
                    nc.vector.tensor_tensor(out=mg, in0=y3all[:, d0:d0 + nd, :],
                                            in1=dxS[:, d0:d0 + nd, :],
                                            op=ALU.mult)
                    mgs.append(mg)
                    if gi == 1:
                        s01 = apool.tile([128, 3, Nc], F32, tag="s01",
                                         name=f"s01_{s}")
                        nc.vector.tensor_tensor(out=s01, in0=mgs[0], in1=mgs[1],
                                                op=ALU.add)
                        t1 = apool.tile([128, Nc], F32, tag="t1", name=f"t1_{s}")
                        nc.vector.tensor_tensor(out=t1, in0=s01[:, 0, :],
                                                in1=s01[:, 1, :], op=ALU.add)
                        t2 = apool.tile([128, Nc], F32, tag="t2", name=f"t2_{s}")
                        nc.vector.tensor_tensor(out=t2, in0=t1, in1=s01[:, 2, :],
                                                op=ALU.add)
                    elif gi == 2:
                        r1 = apool.tile([128, Nc], F32, tag="r1", name=f"r1_{s}")
                        nc.vector.tensor_tensor(out=r1, in0=mgs[2][:, 0, :],
                                                in1=mgs[2][:, 1, :], op=ALU.add)
                        r2 = apool.tile([128, Nc], F32, tag="r2", name=f"r2_{s}")
                        nc.vector.tensor_tensor(out=r2, in0=r1,
                                                in1=mgs[2][:, 2, :], op=ALU.add)
                        v2 = apool.tile([128, Nc], F32, tag="v2", name=f"v2_{s}")
                        nc.vector.tensor_tensor(out=v2, in0=t2, in1=r2,
                                                op=ALU.add)
                # PE fillers for the tail: next stage's dx DMA + L3 bias seeds
                if s + 1 < N_STAGES:
                    emit_bcast(s + 1)
                    emit_l3_bias(s + 1)
                return v2, mgs[3][:, 0, :]

            def tail(s, v2, mg3, coef, zbase):
                """Seed p0(s+1) = W0^T(zbase + coef*v2) + W0^T(mg3') where
                mg3' is host-prescaled by coef; k reconstructed off-crit."""
                w = apool.tile([E, Nc], BF16, tag="w", name=f"w_{s}")
                stt("dve", w, v2, coef, zbase)        # off-tail (v2 early)
                p0s = seed_p0(s + 1, w, mg3)
                k_s = apool.tile([E, Nc], F32,
                                 tag=("k1" if s % 4 == 0 else "ks"),
                                 name=f"k_{s}")
                stt("dve", k_s, mg3, 1.0 / coef, v2)  # off-crit
                return p0s, k_s

            p0s = seed_p0(0, zin, None)
            for j in range(N_STEPS):
                hs = float(h[j])
                last = j == N_STEPS - 1

                v2, mg3 = vf_stage(4 * j + 0, p0s)
                p0s, k1 = tail(4 * j, v2, mg3, hs / 3.0, z)
                zpart3 = apool.tile([E, Nc], F32, tag="zp3", name=f"zp3_{j}")
                stt("dve", zpart3, k1, -hs / 3.0, z)
                zacc = apool.tile([E, Nc], F32, tag="za", name=f"za1_{j}")
                stt("dve", zacc, k1, hs / 8.0, z)

                v2, mg3 = vf_stage(4 * j + 1, p0s)
                p0s, k2 = tail(4 * j + 1, v2, mg3, hs, zpart3)
                u12 = apool.tile([E, Nc], F32, tag="u12", name=f"u12_{j}")
                stt("dve", u12, k2, -1.0, k1)
                zpart4 = apool.tile([E, Nc], F32, tag="zp4", name=f"zp4_{j}")
                stt("dve", zpart4, u12, hs, z)
                zacc2 = apool.tile([E, Nc], F32, tag="za", name=f"za2_{j}")
                stt("dve", zacc2, k2, 3.0 * hs / 8.0, zacc)

                v2, mg3 = vf_stage(4 * j + 2, p0s)
                p0s, k3 = tail(4 * j + 2, v2, mg3, hs, zpart4)
                zacc3 = apool.tile([E, Nc], F32, tag="za", name=f"za3_{j}")
                stt("dve", zacc3, k3, 3.0 * hs / 8.0, zacc2)

                v2, mg3 = vf_stage(4 * j + 3, p0s)
                if not last:
                    p0s, k4 = tail(4 * j + 3, v2, mg3, hs / 8.0, zacc3)
                    znew = apool.tile([E, Nc], F32, tag="z", name=f"z_{j + 1}")
                    stt("dve", znew, k4, hs / 8.0, zacc3)
                else:
                    wl = apool.tile([E, Nc], F32, tag="wl", name="wl")
                    stt("dve", wl, v2, hs / 8.0, zacc3)
                    znew = apool.tile([E, Nc], F32, tag="zfin", name=f"z_{j + 1}")
                    nc.vector.tensor_tensor(out=znew, in0=wl, in1=mg3,
                                            op=ALU.add)
                z = znew

            nc.sync.dma_start(out=d_out[:, :], in_=z)
    nc.finalize()
    return nc


def _b3seg(b3p):
    out = np.zeros((128, 4 * E), np.float32)
    for gi, (d0, nd) in enumerate(DX_GROUPS):
        for i in range(nd):
            out[i, gi * E:(gi + 1) * E] = b3p[(d0 + i) * E:(d0 + i + 1) * E]
    return out


def _sel3(Nc):
    out = np.zeros((128, 3 * Nc), np.float32)
    for i in range(3):
        out[i, i * Nc:(i + 1) * Nc] = 1.0
    return out


def _enable_ldw_opt():
    from concourse import bass_utils as _bu
    if getattr(_bu, "_ldwopt_patched", False):
        return
    _orig = _bu.run_command

    def _run2(argv, **kw):
        argv = ["--enable-ldw-opt=true" if a == "--enable-ldw-opt=false" else a
                for a in argv]
        return _orig(argv, **kw)

    _bu.run_command = _run2
    _bu._ldwopt_patched = True


def _prep_host(t, x, mask, W_embed, b_embed, W0, b0, W1, b1, W2, b2, W3, b3):
    import ml_dtypes
    bf = ml_dtypes.bfloat16

    t = np.asarray(t, np.float32)
    x = np.asarray(x, np.float32)
    mask = np.asarray(mask)
    B, Amax = mask.shape
    N = B * Amax

    C60, h = spline_stage_matrix(t)
    idx = np.flatnonzero(mask.ravel())
    nact = max(1, len(idx))
    Nc = min(512, 4 * ((nact + 4 * N_CORES - 1) // (4 * N_CORES)))
    total = N_CORES * Nc
    pad = np.full(total, idx[0] if len(idx) else 0, dtype=np.int64)
    pad[:len(idx)] = idx
    xp = x.reshape(N, T, D)[pad]

    perm = w3_perm()
    shared = dict(
        b_emb=np.asarray(b_embed, np.float32).reshape(E, 1),
        w0t=np.ascontiguousarray(np.asarray(W0).T).astype(bf),
        w1t=np.ascontiguousarray(np.asarray(W1).T).astype(bf),
        w2t=np.ascontiguousarray(np.asarray(W2).T).astype(bf),
        w3pt=np.ascontiguousarray(np.asarray(W3)[perm].T).astype(bf),
        b012=np.stack([np.asarray(b, np.float32)[m * 128:(m + 1) * 128]
                       for b in (b0, b1, b2) for m in range(4)],
                      axis=1).astype(np.float32),
        b3seg=_b3seg(np.asarray(b3, np.float32)[perm]).astype(bf),
        sel3=_sel3(Nc).astype(bf),
        w_embt=np.concatenate([np.asarray(W_embed, np.float32).T,
                               np.zeros((128 - D, E), np.float32)], 0).astype(bf),
    )
    dx_all = np.einsum("st,ntd->snd", C60, xp.astype(np.float64))  # (60,tot,D)
    for s in range(N_STAGES):
        hs = float(h[s // 4])
        coef = (hs / 3.0, hs, hs, hs / 8.0)[s % 4]
        dx_all[s, :, D - 1] *= coef
    in_maps = []
    for c in range(N_CORES):
        xc = xp[c * Nc:(c + 1) * Nc]
        dxc = dx_all[:, c * Nc:(c + 1) * Nc, :]  # (60, Nc, D)
        dxh = np.ascontiguousarray(dxc.transpose(2, 0, 1).reshape(D, -1))
        in_maps.append(dict(
            dxh=dxh.astype(bf),                        # (10, 60*Nc)
            x_t0=np.concatenate([np.ascontiguousarray(xc[:, 0, :].T),
                                 np.zeros((128 - D, Nc), np.float32)],
                                0).astype(bf),
            **shared,
        ))
    return in_maps, pad, len(idx), Nc, h


def kernel(t, x, mask, W_embed, b_embed, W0, b0, W1, b1, W2, b2, W3, b3):
    global last_results
    from concourse import bass_utils
    if os.environ.get("KERNEL_LDWOPT", "0") == "1":
        _enable_ldw_opt()

    mask = np.asarray(mask)
    B, Amax = mask.shape
    N = B * Amax

    in_maps, pad, nact, Nc, h = _prep_host(
        t, x, mask, W_embed, b_embed, W0, b0, W1, b1, W2, b2, W3, b3)

    nc = build_bass2(Nc, h)
    res = bass_utils.run_bass_kernel_spmd(
        nc, in_maps, core_ids=list(range(N_CORES)))
    last_results = res

    zall = np.concatenate([r["zout"].T for r in res.results], 0)  # (total, E)
    out = np.zeros((N, E), np.float32)
    out[pad[:nact]] = zall[:nact]
    return out.reshape(B, Amax, E)
